# revision 1
# baseline (speedup 1.0000x reference)
"""Trainium2 Bass kernel: 16-member MLP ensemble (1024 -> 256 relu -> 128 relu -> 16 tanh).

Sharding: expert-parallel over the ensemble axis -- 2 members per NeuronCore x 8 cores,
fully independent (no collectives).

Device layout strategy: the PE contracts along the partition dim, so every operand is
pre-arranged host-side with the contraction dim on partitions:
  x   -> [mpc, 128, 8, B]   (x^T tiles: input-feature chunks on partitions)
  W1  -> [mpc, 128, 8, 256] (W1^T: lhsT tiles [K=128, M=256])
  W2  -> [mpc, 128, 2, 128]
  W3  -> [mpc, 128, 16]
Hidden activations stay in SBUF between layers (h1/h2 never touch HBM); the kernel output
is out^T [mpc, 16, B] per core, un-transposed on host.

Layer 1 (the 1024-wide contraction, ~90% of FLOPs and bytes) runs with fp16 x and W1
(fp32 PSUM accumulation) -- halves the dominant HBM stream; rounding error ~3e-4, on par
with fp32r's own error. Layers 2/3 run as float32r (fp32 data, full-rate PE mode for
moving-dim >= 256). Measured on HW: absmax 9.6e-04 / fro-rel 3.7e-04 vs fp32 reference.
"""

import numpy as np

import concourse.bacc as bacc
import concourse.bass as bass
import concourse.mybir as mybir
import concourse.tile as tile
from concourse.bass_utils import run_bass_kernel_spmd
from concourse.tile import add_dep_helper

M, B, Z = 16, 4096, 16
N_CORES = 8
MPC = M // N_CORES          # models per core
D_IN, H1, H2 = 1024, 256, 128
BT = 512                    # batch tile (fp32 moving-operand max / one PSUM bank)
NBT = B // BT
KC1 = D_IN // 128           # contraction chunks, layer 1
KC2 = H1 // 128             # contraction chunks, layer 2
OC1 = H1 // 128             # output chunks, layer 1

F32 = mybir.dt.float32
F32R = mybir.dt.float32r
F16 = mybir.dt.float16
AF = mybir.ActivationFunctionType

_cached = None
last_results = None         # BassKernelResults from the most recent run (for test harness)


def build_bass():
    nc = bacc.Bacc("TRN2", target_bir_lowering=False, debug=False, num_devices=N_CORES)

    xh = nc.dram_tensor("xh", [MPC, 128, KC1, B], F16, kind="ExternalInput")
    w1h = nc.dram_tensor("w1h", [MPC, 128, KC1, H1], F16, kind="ExternalInput")
    b1h = nc.dram_tensor("b1h", [MPC, 128, OC1], F32, kind="ExternalInput")
    w2h = nc.dram_tensor("w2h", [MPC, 128, KC2, H2], F32R, kind="ExternalInput")
    b2h = nc.dram_tensor("b2h", [MPC, 128, 1], F32, kind="ExternalInput")
    w3h = nc.dram_tensor("w3h", [MPC, 128, Z], F32R, kind="ExternalInput")
    b3h = nc.dram_tensor("b3h", [MPC, Z, 1], F32, kind="ExternalInput")
    outh = nc.dram_tensor("outh", [MPC, Z, B], F32, kind="ExternalOutput")

    with tile.TileContext(nc) as tc:
        with (
            tc.tile_pool(name="weights", bufs=1) as wp,
            tc.tile_pool(name="xin", bufs=5) as xp,
            tc.tile_pool(name="hid", bufs=4) as hp,
            tc.tile_pool(name="outs", bufs=4) as op,
            tc.tile_pool(name="ps", bufs=2, space="PSUM") as pp,
            tc.tile_pool(name="ps1p", bufs=4, space="PSUM") as pp1,
            tc.tile_pool(name="ps3p", bufs=1, space="PSUM") as pp3,
            tc.tile_pool(name="warm", bufs=1, space="PSUM") as wpp,
        ):
            # Weight/bias DMAs. w1 of model 0 goes first so the PE can start
            # layer 1 as early as possible; everything else trickles in behind
            # the first x tile on the queue.
            wt = [[None] * 6 for _ in range(MPC)]
            wdmas = []
            w1_0 = wp.tile([128, KC1, H1], F16, name="w1_0", tag="w1_0")
            wdmas.append(nc.sync.dma_start(w1_0[:], w1h[0]))
            wt[0][0] = w1_0
            for m in range(MPC):
                if m > 0:
                    w1m = wp.tile([128, KC1, H1], F16, name=f"w1_{m}", tag=f"w1_{m}")
                    wdmas.append(nc.sync.dma_start(w1m[:], w1h[m]))
                    wt[m][0] = w1m
                w2 = wp.tile([128, KC2, H2], F32R, name=f"w2_{m}", tag=f"w2_{m}")
                wdmas.append(nc.sync.dma_start(w2[:], w2h[m]))
                w3 = wp.tile([128, Z], F32R, name=f"w3_{m}", tag=f"w3_{m}")
                wdmas.append(nc.sync.dma_start(w3[:], w3h[m]))
                b1 = wp.tile([128, OC1], F32, name=f"b1_{m}", tag=f"b1_{m}")
                wdmas.append(nc.sync.dma_start(b1[:], b1h[m]))
                b2 = wp.tile([128, 1], F32, name=f"b2_{m}", tag=f"b2_{m}")
                wdmas.append(nc.sync.dma_start(b2[:], b2h[m]))
                b3 = wp.tile([Z, 1], F32, name=f"b3_{m}", tag=f"b3_{m}")
                wdmas.append(nc.sync.dma_start(b3[:], b3h[m]))
                wt[m][1:] = [w2, w3, b1, b2, b3]

            wps = wpp.tile([128, 16], F32, name="warm_ps", tag="warm_ps")

            def emit_chunk(m, tag, w1, w2, w3, b1, b2, b3, xt, xs, outs, width):
                """One fused 3-layer pass over `width` batch columns.
                xt[:, c, xs] supplies the layer-1 rhs; result stored to outh[m][:, outs]."""
                h1c = []
                for oc in range(OC1):
                    ps1 = pp1.tile([128, width], F32, name=f"ps1_{tag}_{oc}", tag="ps1")
                    for c in range(KC1):
                        nc.tensor.matmul(
                            ps1[:],
                            lhsT=w1[:, c, oc * 128:(oc + 1) * 128],
                            rhs=xt[:, c, xs],
                            start=(c == 0),
                            stop=(c == KC1 - 1),
                        )
                    h1 = hp.tile([128, width], F32R, name=f"h1_{tag}_{oc}", tag="h1")
                    nc.scalar.activation(h1[:], ps1[:], AF.Relu, bias=b1[:, oc:oc + 1])
                    h1c.append(h1)

                ps2 = pp.tile([128, width], F32, name=f"ps2_{tag}", tag="ps2")
                for c in range(KC2):
                    nc.tensor.matmul(
                        ps2[:],
                        lhsT=w2[:, c, :],
                        rhs=h1c[c][:],
                        start=(c == 0),
                        stop=(c == KC2 - 1),
                    )
                h2 = hp.tile([128, width], F32R, name=f"h2_{tag}", tag="h2")
                nc.scalar.activation(h2[:], ps2[:], AF.Relu, bias=b2[:, 0:1])

                ps3 = pp3.tile([Z, width], F32, name=f"ps3_{tag}", tag="ps3")
                nc.tensor.matmul(ps3[:], lhsT=w3[:], rhs=h2[:], start=True, stop=True)
                ot = op.tile([Z, width], F32, name=f"ot_{tag}", tag="ot")
                nc.scalar.activation(ot[:], ps3[:], AF.Tanh, bias=b3[:, 0:1])
                store_eng = nc.scalar if width != BT else nc.gpsimd
                store_eng.dma_start(outh[m][:, outs], ot[:])

            XW = BT               # columns per x DMA (2 MiB transfers)
            for m in range(MPC):
                w1, w2, w3, b1, b2, b3 = wt[m]
                # Weight-touch warmups, per model: the walrus fp32r self-loading
                # matmul has a single sync-wait slot, so no real matmul may wait
                # on both its weight DMA and its rhs producer. Touch each weight
                # tile with a tiny matmul carrying the weight-DMA wait alone.
                nc.tensor.matmul(wps[:], lhsT=w1[:, 0, 0:128],
                                 rhs=w1[:, 0, 0:16], start=True, stop=True)
                nc.tensor.matmul(wps[:], lhsT=w2[:, 0, 0:128],
                                 rhs=w2[:, 0, 0:16], start=True, stop=True)
                nc.tensor.matmul(wps[0:16, :], lhsT=w3[:, 0:16],
                                 rhs=w3[:, 0:16], start=True, stop=True)

                last = MPC - 1 == m
                for tx in range(B // XW):
                    xt = xp.tile([128, KC1, XW], F16, name=f"x_{m}_{tx}", tag="xt")
                    x_ap = xh[m][:, :, tx * XW:(tx + 1) * XW]
                    tail = last and tx == B // XW - 1
                    if not tail:
                        xdma = nc.sync.dma_start(xt[:], x_ap)
                        if m == 0 and tx == 0:
                            # Keeps the first bulk x chunk at the queue head with
                            # the small weight DMAs immediately behind it.
                            for wd in wdmas:
                                add_dep_helper(wd.ins, xdma.ins, sync=False,
                                               reason="weights before x bulk stream")
                        emit_chunk(m, f"{m}_{tx}", w1, w2, w3, b1, b2, b3,
                                   xt, slice(0, XW), slice(tx * XW, (tx + 1) * XW), XW)
                    else:
                        # Final chunk: split into halves so the tail drain
                        # overlaps the last x bytes still in flight.
                        hw_ = XW // 2
                        for h in range(2):
                            hs = slice(h * hw_, (h + 1) * hw_)
                            # split each half k-wise too: layer-1 accumulation of
                            # chunks 0-3 starts while chunks 4-7 are in flight
                            nc.sync.dma_start(xt[:, 0:KC1 // 2, hs],
                                              x_ap[:, 0:KC1 // 2, hs])
                            nc.sync.dma_start(xt[:, KC1 // 2:KC1, hs],
                                              x_ap[:, KC1 // 2:KC1, hs])
                            emit_chunk(m, f"{m}_{tx}_h{h}", w1, w2, w3, b1, b2, b3,
                                       xt, slice(h * hw_, (h + 1) * hw_),
                                       slice(tx * XW + h * hw_, tx * XW + (h + 1) * hw_), hw_)

    nc.compile()
    return nc


def make_in_maps(x, W1, b1, W2, b2, W3, b3):
    """Host-side shard + layout prep. Returns one input map per core."""
    xb = np.asarray(x, dtype=np.float32).reshape(M, B, D_IN)
    W1 = np.asarray(W1, dtype=np.float32)
    W2 = np.asarray(W2, dtype=np.float32)
    W3 = np.asarray(W3, dtype=np.float32)
    b1 = np.asarray(b1, dtype=np.float32)
    b2 = np.asarray(b2, dtype=np.float32)
    b3 = np.asarray(b3, dtype=np.float32)

    in_maps = []
    for core in range(N_CORES):
        sl = slice(core * MPC, (core + 1) * MPC)
        # x: [mpc,B,1024] -> i=(c,p) -> [mpc,128,KC1,B]
        xh = np.ascontiguousarray(
            xb[sl].reshape(MPC, B, KC1, 128).transpose(0, 3, 2, 1)).astype(np.float16)
        # W1: [mpc,256,1024] -> [mpc,128,KC1,256]
        w1h = np.ascontiguousarray(
            W1[sl].reshape(MPC, H1, KC1, 128).transpose(0, 3, 2, 1)).astype(np.float16)
        # W2: [mpc,128,256] -> [mpc,128,KC2,128]
        w2h = np.ascontiguousarray(
            W2[sl].reshape(MPC, H2, KC2, 128).transpose(0, 3, 2, 1))
        # W3: [mpc,16,128] -> [mpc,128,16]
        w3h = np.ascontiguousarray(W3[sl].transpose(0, 2, 1))
        b1t = np.ascontiguousarray(b1[sl].reshape(MPC, OC1, 128).transpose(0, 2, 1))
        b2t = np.ascontiguousarray(b2[sl].reshape(MPC, 128, 1))
        b3t = np.ascontiguousarray(b3[sl].reshape(MPC, Z, 1))
        in_maps.append({
            "xh": xh, "w1h": w1h, "b1h": b1t,
            "w2h": w2h, "b2h": b2t, "w3h": w3h, "b3h": b3t,
        })
    return in_maps


def kernel(x, W1, b1, W2, b2, W3, b3):
    global _cached, last_results
    if _cached is None:
        _cached = build_bass()
    nc = _cached

    in_maps = make_in_maps(x, W1, b1, W2, b2, W3, b3)
    res = run_bass_kernel_spmd(nc, in_maps, list(range(N_CORES)))
    last_results = res

    # outh per core: [MPC, Z, B] -> full output [M, B, Z]
    parts = [r["outh"] for r in res.results]
    out_t = np.concatenate(parts, axis=0)             # [M, Z, B]
    return np.ascontiguousarray(out_t.transpose(0, 2, 1)).astype(np.float32)



# revision 2
# speedup vs baseline: 1.0028x; 1.0028x over previous
"""Trainium2 Bass kernel: 16-member MLP ensemble (1024 -> 256 relu -> 128 relu -> 16 tanh).

Sharding: expert-parallel, 2 models per core x 8 cores, no collectives.

Layer 1 (90% of FLOPs) runs as fp8-e4m3 DoubleRow matmuls (2 k-tiles per
instruction, 0.5 cyc/col on the PE) with residual compensation:
  W' = e4m3(32*W1), Wl = e4m3(32*W1 - W'); x_hi = e4m3(x), x_lo = e4m3(x - x_hi).
  Corrected k-chunks (NC_CORR=6): (W'_c, W'_c) . (x_hi_c, x_lo_c)      [DR1]
  Per corrected chunk-pair:       (Wl_c0, Wl_c1) . (x_hi_c0, x_hi_c1)  [DR2]
  Uncorrected chunks 6,7:         (W'_6, W'_7) . (x_hi_6, x_hi_7)      [DR3]
                                  (Wl_6, Wl_7) . (x_hi_6, x_hi_7)      [DR4]
  DR2/DR3/DR4 reuse x_hi bytes via strided APs (x is 1.75 B/elem); DR1's
  duplicated W' pair is a stride-0 broadcast AP (verified on HW), so W1 is
  sent once. relu(32(W1 x + b1)) = 32 relu(W1 x + b1): the 32x folds into
  W2/32 host-side. Layers 2/3 fp16. L3 outputs for up to 4 batch tiles are
  packed into one PSUM bank at partition strips 0/32/64/96 (one tanh + one
  store per group; output layout [mpc, grp, 128, BT] fp16, host unpacks).

Schedule: every DMA is issued upfront (all x tiles fit in SBUF, no WAR),
spread over the SP and Pool(SWDGE) queues in strict consumption order
(a DMA holds its queue for dge+transfer+sem-prop, and the shared DMA-engine
resource grants FIFO by request time). Model-1 weights go on the ACT queue
mid-stream via program position. The PE stream is software-pipelined
(L1 of bt+1 before L2/L3 of bt) because any PE idle gap resets the cost
model's p-state ramp (matmuls run 2-3.7x slower for 3us after a gap).

Host-sim rel err: 1.31e-2 (gate 2e-2); HW matches (1.307e-2).
"""

import numpy as np
import ml_dtypes

import concourse.bacc as bacc
import concourse.bass as bass
import concourse.mybir as mybir
import concourse.tile as tile
from concourse.bass_utils import run_bass_kernel_spmd

M, B, Z = 16, 4096, 16
N_CORES = 8
MPC = M // N_CORES
D_IN, H1, H2 = 1024, 256, 128
BT = 512
NBT = B // BT
KC = D_IN // 128
NC_CORR = 6
NPAIR = NC_CORR // 2
OC1 = H1 // 128
XR = 1 + NC_CORR            # x row-pairs: 0=(hi6,hi7), 1..6=(hi_c,lo_c)
NW = 4 + NC_CORR + 2 * NPAIR  # w1 rows: 0-1=(W'6,W'7) 2-3=(Wl6,Wl7) 4-9=W'c 10-15=Wl
NGRP = 3
BB = 2 * H2 + Z               # bias-column base inside the merged wb tensor

F32 = mybir.dt.float32
F16 = mybir.dt.float16
F8 = mybir.dt.float8e4
AF = mybir.ActivationFunctionType
DR = mybir.MatmulPerfMode.DoubleRow
E4 = ml_dtypes.float8_e4m3

GROUPS = [[4, 4] if m < MPC - 1 else [4, 3, 1] for m in range(MPC)]

_cached = None
last_results = None


def build_bass():
    nc = bacc.Bacc("TRN2", target_bir_lowering=False, debug=False, num_devices=N_CORES)

    xh = nc.dram_tensor("xh", [MPC, 128, XR, 2, B], F8, kind="ExternalInput")
    w1h = nc.dram_tensor("w1h", [MPC, 128, NW, H1], F8, kind="ExternalInput")
    wbh = nc.dram_tensor("wbh", [MPC, 128, 2 * H2 + Z], F16,
                         kind="ExternalInput")
    bfh = nc.dram_tensor("bfh", [MPC, 128, OC1 + 2], F32, kind="ExternalInput")
    outh = nc.dram_tensor("outh", [MPC, NGRP, 128, BT], F16, kind="ExternalOutput")

    with tile.TileContext(nc) as tc:
        with (
            tc.tile_pool(name="static", bufs=1) as sp,
            tc.tile_pool(name="hid", bufs=6) as hp,
            tc.tile_pool(name="outs", bufs=2) as op,
            tc.tile_pool(name="ps1p", bufs=4, space="PSUM") as pp1,
            tc.tile_pool(name="ps2p", bufs=2, space="PSUM") as pp2,
            tc.tile_pool(name="ps3p", bufs=1, space="PSUM") as pp3,
            tc.tile_pool(name="warm", bufs=1, space="PSUM") as wpp,
        ):
            wt = []
            for m in range(MPC):
                wt.append({
                    "w1": sp.tile([128, NW, H1], F8, name=f"w1_{m}"),
                    "wb": sp.tile([128, 2 * H2 + Z], F16, name=f"wb_{m}"),
                    "bf": sp.tile([128, OC1 + 2], F32, name=f"bf_{m}"),
                })
            xtiles = {}
            for m in range(MPC):
                for bt in range(NBT):
                    xtiles[(m, bt)] = sp.tile([128, XR, 2, BT], F8,
                                              name=f"x_{m}_{bt}")

            # ---- DMA issue plan ----
            def dma_x(eng, m, bt, rows=None):
                t = xtiles[(m, bt)]
                cols = slice(bt * BT, (bt + 1) * BT)
                if rows is None:
                    return eng.dma_start(t[:], xh[m][:, :, :, cols])
                return eng.dma_start(t[:, rows, :, :], xh[m][:, rows, :, cols])

            # Single-queue supply: same-queue DMAs stream back-to-back on the
            # DMA engines (dge/sem-prop overheads pipeline with neighbours),
            # and in-queue order is the only reliable grant order. So the
            # entire supply rides the SP queue in exact consumption order;
            # only the output stores (tiny, latency-tolerant) use the ACT
            # queue. Total transfer time ~44us < PE ~48us, so the stream
            # stays ahead of compute from bt1 on.
            nc.sync.dma_start(wt[0]["w1"][:, 0:4, :], w1h[0][:, 0:4])
            dma_x(nc.sync, 0, 0, rows=0)
            nc.sync.dma_start(wt[0]["w1"][:, 4:NW, :], w1h[0][:, 4:NW])
            dma_x(nc.sync, 0, 0, rows=slice(1, 3))
            dma_x(nc.sync, 0, 0, rows=slice(3, 5))
            dma_x(nc.sync, 0, 0, rows=slice(5, XR))
            dma_x(nc.sync, 0, 1)
            nc.sync.dma_start(wt[0]["wb"][:], wbh[0])
            nc.sync.dma_start(wt[0]["bf"][:], bfh[0])
            dma_x(nc.sync, 0, 2)
            dma_x(nc.sync, 0, 3)
            dma_x(nc.sync, 0, 4)
            dma_x(nc.sync, 0, 5)
            dma_x(nc.sync, 0, 6)
            dma_x(nc.sync, 0, 7)
            nc.sync.dma_start(wt[1]["w1"][:], w1h[1])
            nc.sync.dma_start(wt[1]["wb"][:], wbh[1])
            nc.sync.dma_start(wt[1]["bf"][:], bfh[1])
            for bt in range(7):
                dma_x(nc.sync, 1, bt)
            dma_x(nc.sync, 1, 7, rows=slice(0, 5))
            dma_x(nc.sync, 1, 7, rows=slice(5, XR))

            wps = wpp.tile([128, 16], F32, name="warm_ps")

            def warm_w1a(m):
                nc.tensor.matmul(wps[:], lhsT=wt[m]["w1"][:, 0, 0:128],
                                 rhs=wt[m]["w1"][:, 0, 0:16], start=True, stop=True)

            def warm_w1b(m):
                nc.tensor.matmul(wps[:], lhsT=wt[m]["w1"][:, 4, 0:128],
                                 rhs=wt[m]["w1"][:, 4, 0:16], start=True, stop=True)

            def warm_wb(m):
                nc.tensor.matmul(wps[:], lhsT=wt[m]["wb"][:, 0:128],
                                 rhs=wt[m]["wb"][:, 0:16], start=True, stop=True)

            act_cooldown = 0

            def l1_pass(m, bt, tag, after_dr34=None):
                t = wt[m]
                xt = xtiles[(m, bt)]

                h1c = []
                for oc in range(OC1):
                    osl = slice(oc * 128, (oc + 1) * 128)
                    ps1 = pp1.tile([128, BT], F32, name=f"ps1_{tag}_{oc}",
                                   tag="ps1")

                    def dr1(c, stop=False):
                        nc.tensor.matmul(
                            ps1[:],
                            lhsT=t["w1"][:, 4 + c:5 + c, osl]
                                .broadcast_to([128, 2, 128]),
                            rhs=xt[:, 1 + c, :, :],
                            start=False, stop=stop, perf_mode=DR)

                    def dr2(p, stop=False):
                        nc.tensor.matmul(
                            ps1[:], lhsT=t["w1"][:, 10 + 2 * p:12 + 2 * p, osl],
                            rhs=xt[:, 1 + 2 * p:3 + 2 * p, 0, :],
                            start=False, stop=stop, perf_mode=DR)

                    # order consumes x rows 0..4 before rows 5..6 (piece-split
                    # head/tail tiles), and w1 rows 0-3 before the rest
                    nc.tensor.matmul(ps1[:], lhsT=t["w1"][:, 0:2, osl],
                                     rhs=xt[:, 0, :, :],
                                     start=True, stop=False, perf_mode=DR)
                    nc.tensor.matmul(ps1[:], lhsT=t["w1"][:, 2:4, osl],
                                     rhs=xt[:, 0, :, :],
                                     start=False, stop=False, perf_mode=DR)
                    if after_dr34 is not None:
                        after_dr34()
                        after_dr34 = None
                    dr1(0)
                    dr1(1)
                    dr1(2)
                    dr2(0)
                    dr1(3)
                    dr2(1)
                    dr1(4)
                    dr1(5)
                    dr2(2, stop=True)

                    h1 = hp.tile([128, BT], F16, name=f"h1_{tag}_{oc}", tag="h1")
                    nc.vector.tensor_scalar(h1[:], ps1[:],
                                            t["bf"][:, oc:oc + 1],
                                            0.0, mybir.AluOpType.add,
                                            mybir.AluOpType.max)
                    h1c.append(h1)
                return h1c

            def l23_pass(m, bt, h1c, tag):
                nonlocal act_cooldown
                t = wt[m]
                ps2 = pp2.tile([128, BT], F32, name=f"ps2_{tag}", tag="ps2")
                for c in range(2):
                    nc.tensor.matmul(ps2[:], lhsT=t["wb"][:, c * H2:(c + 1) * H2],
                                     rhs=h1c[c][:], start=(c == 0), stop=(c == 1))
                h2 = hp.tile([128, BT], F16, name=f"h2_{tag}", tag="h2")
                if act_cooldown > 0:
                    act_cooldown -= 1
                    nc.vector.tensor_scalar(h2[:], ps2[:],
                                            t["bf"][:, OC1:OC1 + 1], 0.0,
                                            mybir.AluOpType.add,
                                            mybir.AluOpType.max)
                else:
                    nc.scalar.activation(h2[:], ps2[:], AF.Relu,
                                         bias=t["bf"][:, OC1:OC1 + 1])
                return h2

            ginfo = {}
            for m in range(MPC):
                s = 0
                for g, gsz in enumerate(GROUPS[m]):
                    for k in range(gsz):
                        ginfo[(m, s + k)] = (g, k, gsz)
                    s += gsz

            grp_tiles = {}

            def finish(m, bt, h1c):
                nonlocal act_cooldown
                tag = f"{m}_{bt}"
                g, k, gsz = ginfo[(m, bt)]
                if k == 0:
                    grp_tiles[(m, g)] = pp3.tile([128, BT], F32,
                                                 name=f"ps3_{m}_{g}", tag="ps3")
                ps3 = grp_tiles[(m, g)]
                h2 = l23_pass(m, bt, h1c, tag)
                nc.tensor.matmul(ps3[32 * k:32 * k + Z, :],
                                 lhsT=wt[m]["wb"][:, 2 * H2:2 * H2 + Z],
                                 rhs=h2[:], start=True, stop=True,
                                 tile_position=(0, 32 * k))
                if k == gsz - 1:
                    rows = 32 * (gsz - 1) + Z
                    ot = op.tile([128, BT], F16, name=f"ot_{m}_{g}", tag="ot")
                    nc.scalar.activation(ot[0:rows, :], ps3[0:rows, :], AF.Tanh,
                                         bias=wt[m]["bf"][0:rows,
                                                          OC1 + 1:OC1 + 2])
                    nc.scalar.dma_start(outh[m, g][0:rows, :], ot[0:rows, :])
                    act_cooldown = 2

            # software-pipelined PE order: L1(bt+1) before L2/L3(bt).
            # Warmup matmuls (absorbing weight-DMA waits) are interleaved so
            # the in-order PE queue never blocks on a weight that arrives
            # later than the x rows it could already be processing.
            warm_w1a(0)
            seq = [(m, bt) for m in range(MPC) for bt in range(NBT)]
            pend = None
            for m, bt in seq:
                if m == 1 and bt == 0:
                    warm_w1a(1)
                    warm_w1b(1)
                    warm_wb(1)
                h1c = l1_pass(m, bt, f"{m}_{bt}",
                              after_dr34=(lambda: warm_w1b(0))
                              if (m, bt) == (0, 0) else None)
                if pend is None:
                    warm_wb(0)
                else:
                    finish(*pend)
                pend = (m, bt, h1c)
            finish(*pend)

    nc.compile()
    return nc


def make_in_maps(x, W1, b1, W2, b2, W3, b3):
    xb = np.asarray(x, dtype=np.float32).reshape(M, B, D_IN)
    W1 = np.asarray(W1, dtype=np.float32)
    W2 = np.asarray(W2, dtype=np.float32)
    W3 = np.asarray(W3, dtype=np.float32)
    b1 = np.asarray(b1, dtype=np.float32)
    b2 = np.asarray(b2, dtype=np.float32)
    b3 = np.asarray(b3, dtype=np.float32)

    in_maps = []
    for core in range(N_CORES):
        sl = slice(core * MPC, (core + 1) * MPC)
        xr = np.ascontiguousarray(
            xb[sl].reshape(MPC, B, KC, 128).transpose(0, 3, 2, 1))
        x_hi = xr.astype(E4)
        x_lo = (xr - x_hi.astype(np.float32)).astype(E4)
        xA = np.empty((MPC, 128, XR, 2, B), dtype=E4)
        xA[:, :, 0, 0, :] = x_hi[:, :, NC_CORR, :]
        xA[:, :, 0, 1, :] = x_hi[:, :, NC_CORR + 1, :]
        for c in range(NC_CORR):
            xA[:, :, 1 + c, 0, :] = x_hi[:, :, c, :]
            xA[:, :, 1 + c, 1, :] = x_lo[:, :, c, :]

        w1r = np.ascontiguousarray(
            (32.0 * W1[sl]).reshape(MPC, H1, KC, 128).transpose(0, 3, 2, 1))
        w_hi = w1r.astype(E4)
        w_lo = (w1r - w_hi.astype(np.float32)).astype(E4)
        w1A = np.empty((MPC, 128, NW, H1), dtype=E4)
        w1A[:, :, 0, :] = w_hi[:, :, NC_CORR, :]
        w1A[:, :, 1, :] = w_hi[:, :, NC_CORR + 1, :]
        w1A[:, :, 2, :] = w_lo[:, :, NC_CORR, :]
        w1A[:, :, 3, :] = w_lo[:, :, NC_CORR + 1, :]
        for c in range(NC_CORR):
            w1A[:, :, 4 + c, :] = w_hi[:, :, c, :]
        for p in range(NPAIR):
            w1A[:, :, 10 + 2 * p, :] = w_lo[:, :, 2 * p, :]
            w1A[:, :, 11 + 2 * p, :] = w_lo[:, :, 2 * p + 1, :]

        wb = np.zeros((MPC, 128, 2 * H2 + Z), dtype=np.float16)
        w2t = (W2[sl] / 32.0).reshape(MPC, H2, 2, 128).transpose(0, 3, 2, 1)
        wb[:, :, 0:H2] = w2t[:, :, 0, :]
        wb[:, :, H2:2 * H2] = w2t[:, :, 1, :]
        wb[:, :, 2 * H2:BB] = W3[sl].transpose(0, 2, 1)
        bf = np.zeros((MPC, 128, OC1 + 2), dtype=np.float32)
        bf[:, :, 0:OC1] = (32.0 * b1[sl]).reshape(MPC, OC1, 128).transpose(0, 2, 1)
        bf[:, :, OC1] = b2[sl]
        for k in range(4):
            bf[:, 32 * k:32 * k + Z, OC1 + 1] = b3[sl]

        in_maps.append({"xh": xA, "w1h": w1A, "wbh": wb, "bfh": bf})
    return in_maps


def kernel(x, W1, b1, W2, b2, W3, b3):
    global _cached, last_results
    if _cached is None:
        _cached = build_bass()
    nc = _cached

    in_maps = make_in_maps(x, W1, b1, W2, b2, W3, b3)
    res = run_bass_kernel_spmd(nc, in_maps, list(range(N_CORES)))
    last_results = res

    out = np.empty((M, B, Z), dtype=np.float32)
    for core in range(N_CORES):
        oh = res.results[core]["outh"]
        for m in range(MPC):
            gm = core * MPC + m
            s = 0
            for g, gsz in enumerate(GROUPS[m]):
                for k in range(gsz):
                    bt = s + k
                    out[gm, bt * BT:(bt + 1) * BT, :] = (
                        oh[m, g, 32 * k:32 * k + Z, :].T.astype(np.float32))
                s += gsz
    return out


# revision 3
# speedup vs baseline: 1.0218x; 1.0189x over previous
"""Trainium2 Bass kernel: 16-member MLP ensemble (1024 -> 256 relu -> 128 relu -> 16 tanh).

Sharding: expert-parallel, 2 models per core x 8 cores, no collectives.

Layer 1 (90% of FLOPs) runs as fp8-e4m3 DoubleRow matmuls (2 k-tiles per
instruction, 0.5 cyc/col on the PE) with residual compensation:
  W' = e4m3(32*W1), Wl = e4m3(32*W1 - W'); x_hi = e4m3(x), x_lo = e4m3(x - x_hi).
  Corrected k-chunks (NC_CORR=6): (W'_c, W'_c) . (x_hi_c, x_lo_c)      [DR1]
  Per corrected chunk-pair:       (Wl_c0, Wl_c1) . (x_hi_c0, x_hi_c1)  [DR2]
  Uncorrected chunks 6,7:         (W'_6, W'_7) . (x_hi_6, x_hi_7)      [DR3]
                                  (Wl_6, Wl_7) . (x_hi_6, x_hi_7)      [DR4]
  DR2/DR3/DR4 reuse x_hi bytes via strided APs (x is 1.75 B/elem); DR1's
  duplicated W' pair is a stride-0 broadcast AP (verified on HW), so W1 is
  sent once. relu(32(W1 x + b1)) = 32 relu(W1 x + b1): the 32x folds into
  W2/32 host-side. Layers 2/3 fp16. L3 outputs for up to 4 batch tiles are
  packed into one PSUM bank at partition strips 0/32/64/96 (one tanh + one
  store per group; output layout [mpc, grp, 128, BT] fp16, host unpacks).

Schedule: every DMA is issued upfront (all x tiles fit in SBUF, no WAR),
spread over the SP and Pool(SWDGE) queues in strict consumption order
(a DMA holds its queue for dge+transfer+sem-prop, and the shared DMA-engine
resource grants FIFO by request time). Model-1 weights go on the ACT queue
mid-stream via program position. The PE stream is software-pipelined
(L1 of bt+1 before L2/L3 of bt) because any PE idle gap resets the cost
model's p-state ramp (matmuls run 2-3.7x slower for 3us after a gap).

Host-sim rel err: 1.31e-2 (gate 2e-2); HW matches (1.307e-2).
"""

import numpy as np
import ml_dtypes

import concourse.bacc as bacc
import concourse.bass as bass
import concourse.mybir as mybir
import concourse.tile as tile
from concourse.bass_utils import run_bass_kernel_spmd

M, B, Z = 16, 4096, 16
N_CORES = 8
MPC = M // N_CORES
D_IN, H1, H2 = 1024, 256, 128
BT = 512
NBT = B // BT
KC = D_IN // 128
NC_CORR = 6
NPAIR = NC_CORR // 2
OC1 = H1 // 128
XR = 1 + NC_CORR            # x row-pairs: 0=(hi6,hi7), 1..6=(hi_c,lo_c)
NW = 4 + NC_CORR + 2 * NPAIR  # w1 rows: 0-1=(W'6,W'7) 2-3=(Wl6,Wl7) 4-9=W'c 10-15=Wl
NGRP = 3
BB = 2 * H2 + Z               # bias-column base inside the merged wb tensor

F32 = mybir.dt.float32
F16 = mybir.dt.float16
F8 = mybir.dt.float8e4
AF = mybir.ActivationFunctionType
DR = mybir.MatmulPerfMode.DoubleRow
E4 = ml_dtypes.float8_e4m3

GROUPS = [[4, 4] if m < MPC - 1 else [4, 3, 1] for m in range(MPC)]

_cached = None
last_results = None


def build_bass():
    nc = bacc.Bacc("TRN2", target_bir_lowering=False, debug=False, num_devices=N_CORES)

    xh = nc.dram_tensor("xh", [MPC, 128, XR, 2, B], F8, kind="ExternalInput")
    w1h = nc.dram_tensor("w1h", [MPC, 128, NW, H1], F8, kind="ExternalInput")
    wbh = nc.dram_tensor("wbh", [MPC, 128, 2 * H2 + Z], F16,
                         kind="ExternalInput")
    bfh = nc.dram_tensor("bfh", [MPC, 128, OC1 + 2], F32, kind="ExternalInput")
    outh = nc.dram_tensor("outh", [MPC, NGRP, 128, BT], F16, kind="ExternalOutput")

    with tile.TileContext(nc) as tc:
        with (
            tc.tile_pool(name="static", bufs=1) as sp,
            tc.tile_pool(name="hid", bufs=6) as hp,
            tc.tile_pool(name="outs", bufs=2) as op,
            tc.tile_pool(name="ps1p", bufs=4, space="PSUM") as pp1,
            tc.tile_pool(name="ps2p", bufs=2, space="PSUM") as pp2,
            tc.tile_pool(name="ps3p", bufs=1, space="PSUM") as pp3,
            tc.tile_pool(name="warm", bufs=1, space="PSUM") as wpp,
        ):
            wt = []
            for m in range(MPC):
                wt.append({
                    "w1": sp.tile([128, NW, H1], F8, name=f"w1_{m}"),
                    "wb": sp.tile([128, 2 * H2 + Z], F16, name=f"wb_{m}"),
                    "bf": sp.tile([128, OC1 + 2], F32, name=f"bf_{m}"),
                })
            xtiles = {}
            for m in range(MPC):
                for bt in range(NBT):
                    xtiles[(m, bt)] = sp.tile([128, XR, 2, BT], F8,
                                              name=f"x_{m}_{bt}")

            # ---- DMA issue plan ----
            def dma_x(eng, m, bt, rows=None):
                t = xtiles[(m, bt)]
                cols = slice(bt * BT, (bt + 1) * BT)
                if rows is None:
                    return eng.dma_start(t[:], xh[m][:, :, :, cols])
                return eng.dma_start(t[:, rows, :, :], xh[m][:, rows, :, cols])

            # Single-queue supply: same-queue DMAs stream back-to-back on the
            # DMA engines (dge/sem-prop overheads pipeline with neighbours),
            # and in-queue order is the only reliable grant order. So the
            # entire supply rides the SP queue in exact consumption order;
            # only the output stores (tiny, latency-tolerant) use the ACT
            # queue. Total transfer time ~44us < PE ~48us, so the stream
            # stays ahead of compute from bt1 on.
            nc.sync.dma_start(wt[0]["w1"][:, 0:4, :], w1h[0][:, 0:4])
            dma_x(nc.sync, 0, 0, rows=0)
            nc.sync.dma_start(wt[0]["w1"][:, 4:NW, :], w1h[0][:, 4:NW])
            dma_x(nc.sync, 0, 0, rows=slice(1, 3))
            dma_x(nc.sync, 0, 0, rows=slice(3, 5))
            dma_x(nc.sync, 0, 0, rows=slice(5, XR))
            dma_x(nc.sync, 0, 1)
            nc.sync.dma_start(wt[0]["wb"][:], wbh[0])
            nc.sync.dma_start(wt[0]["bf"][:], bfh[0])
            dma_x(nc.sync, 0, 2)
            dma_x(nc.sync, 0, 3)
            dma_x(nc.sync, 0, 4)
            dma_x(nc.sync, 0, 5)
            dma_x(nc.sync, 0, 6)
            dma_x(nc.sync, 0, 7)
            nc.sync.dma_start(wt[1]["w1"][:], w1h[1])
            nc.sync.dma_start(wt[1]["wb"][:], wbh[1])
            nc.sync.dma_start(wt[1]["bf"][:], bfh[1])
            for bt in range(7):
                dma_x(nc.sync, 1, bt)
            dma_x(nc.sync, 1, 7, rows=slice(0, 5))
            dma_x(nc.sync, 1, 7, rows=slice(5, XR))

            wps = wpp.tile([128, 16], F32, name="warm_ps")

            def warm_w1a(m):
                nc.tensor.matmul(wps[:], lhsT=wt[m]["w1"][:, 0, 0:128],
                                 rhs=wt[m]["w1"][:, 0, 0:16], start=True, stop=True)

            def warm_w1b(m):
                nc.tensor.matmul(wps[:], lhsT=wt[m]["w1"][:, 4, 0:128],
                                 rhs=wt[m]["w1"][:, 4, 0:16], start=True, stop=True)

            def warm_wb(m):
                nc.tensor.matmul(wps[:], lhsT=wt[m]["wb"][:, 0:128],
                                 rhs=wt[m]["wb"][:, 0:16], start=True, stop=True)

            def _drs(t, xt, ps1, osl, which):
                def dr1(c, stop=False):
                    nc.tensor.matmul(
                        ps1[:],
                        lhsT=t["w1"][:, 4 + c:5 + c, osl]
                            .broadcast_to([128, 2, 128]),
                        rhs=xt[:, 1 + c, :, :],
                        start=False, stop=stop, perf_mode=DR)

                def dr2(p, stop=False):
                    nc.tensor.matmul(
                        ps1[:], lhsT=t["w1"][:, 10 + 2 * p:12 + 2 * p, osl],
                        rhs=xt[:, 1 + 2 * p:3 + 2 * p, 0, :],
                        start=False, stop=stop, perf_mode=DR)

                if which in ("open", "all"):
                    nc.tensor.matmul(ps1[:], lhsT=t["w1"][:, 0:2, osl],
                                     rhs=xt[:, 0, :, :],
                                     start=True, stop=False, perf_mode=DR)
                    nc.tensor.matmul(ps1[:], lhsT=t["w1"][:, 2:4, osl],
                                     rhs=xt[:, 0, :, :],
                                     start=False, stop=False, perf_mode=DR)
                if which in ("open", "all", "rest"):
                    dr1(0)
                    dr1(1)
                    dr1(2)
                    dr2(0)
                    dr1(3)
                    dr2(1)
                if which in ("close", "all"):
                    dr1(4)
                    dr1(5)
                    dr2(2, stop=True)

            def l1_open(m, bt, tag, after_dr34=None):
                """DR matmuls touching x rows 0..4 only; groups left open."""
                t = wt[m]
                xt = xtiles[(m, bt)]
                ps1s = []
                for oc in range(OC1):
                    ps1 = pp1.tile([128, BT], F32, name=f"ps1_{tag}_{oc}",
                                   tag="ps1")
                    _drs(t, xt, ps1, slice(oc * 128, (oc + 1) * 128), "open")
                    if after_dr34 is not None:
                        after_dr34()
                        after_dr34 = None
                    ps1s.append(ps1)
                return ps1s

            def l1_close(m, bt, tag, ps1s):
                t = wt[m]
                xt = xtiles[(m, bt)]
                h1c = []
                for oc in range(OC1):
                    _drs(t, xt, ps1s[oc], slice(oc * 128, (oc + 1) * 128),
                         "close")
                    h1 = hp.tile([128, BT], F16, name=f"h1_{tag}_{oc}", tag="h1")
                    nc.vector.tensor_scalar(h1[:], ps1s[oc][:],
                                            t["bf"][:, oc:oc + 1],
                                            0.0, mybir.AluOpType.add,
                                            mybir.AluOpType.max)
                    h1c.append(h1)
                return h1c

            def l1_pass(m, bt, tag, after_dr34=None, relu_split=False):
                t = wt[m]
                xt = xtiles[(m, bt)]
                h1c = []
                for oc in range(OC1):
                    osl = slice(oc * 128, (oc + 1) * 128)
                    ps1 = pp1.tile([128, BT], F32, name=f"ps1_{tag}_{oc}",
                                   tag="ps1")
                    if after_dr34 is not None:
                        # emit rows-0 matmuls first, then the hook
                        nc.tensor.matmul(ps1[:], lhsT=t["w1"][:, 0:2, osl],
                                         rhs=xt[:, 0, :, :],
                                         start=True, stop=False, perf_mode=DR)
                        nc.tensor.matmul(ps1[:], lhsT=t["w1"][:, 2:4, osl],
                                         rhs=xt[:, 0, :, :],
                                         start=False, stop=False, perf_mode=DR)
                        after_dr34()
                        after_dr34 = None
                        _drs(t, xt, ps1, osl, "rest")
                        _drs(t, xt, ps1, osl, "close")
                    else:
                        _drs(t, xt, ps1, osl, "all")
                    h1 = hp.tile([128, BT], F16, name=f"h1_{tag}_{oc}", tag="h1")
                    if relu_split and oc == 0:
                        # final tile: put one relu on ACT so the two do not
                        # serialize on DVE during the pipeline drain
                        nc.scalar.activation(h1[:], ps1[:], AF.Relu,
                                             bias=t["bf"][:, oc:oc + 1])
                    else:
                        nc.vector.tensor_scalar(h1[:], ps1[:],
                                                t["bf"][:, oc:oc + 1],
                                                0.0, mybir.AluOpType.add,
                                                mybir.AluOpType.max)
                    h1c.append(h1)
                return h1c

            after_tanh = [False]  # previous finish() emitted a group tanh

            def l23_pass(m, bt, h1c, tag):
                t = wt[m]
                ps2 = pp2.tile([128, BT], F32, name=f"ps2_{tag}", tag="ps2")
                for c in range(2):
                    nc.tensor.matmul(ps2[:], lhsT=t["wb"][:, c * H2:(c + 1) * H2],
                                     rhs=h1c[c][:], start=(c == 0), stop=(c == 1))
                h2 = hp.tile([128, BT], F16, name=f"h2_{tag}", tag="h2")
                if after_tanh[0]:
                    # the ACT queue is still busy with the group tanh; DVE
                    # keeps the relu (and the following L3) off that latency
                    nc.vector.tensor_scalar(h2[:], ps2[:],
                                            t["bf"][:, OC1:OC1 + 1], 0.0,
                                            mybir.AluOpType.add,
                                            mybir.AluOpType.max)
                else:
                    nc.scalar.activation(h2[:], ps2[:], AF.Relu,
                                         bias=t["bf"][:, OC1:OC1 + 1])
                after_tanh[0] = False
                return h2

            ginfo = {}
            for m in range(MPC):
                s = 0
                for g, gsz in enumerate(GROUPS[m]):
                    for k in range(gsz):
                        ginfo[(m, s + k)] = (g, k, gsz)
                    s += gsz

            grp_tiles = {}

            def finish(m, bt, h1c):
                tag = f"{m}_{bt}"
                g, k, gsz = ginfo[(m, bt)]
                if k == 0:
                    grp_tiles[(m, g)] = pp3.tile([128, BT], F32,
                                                 name=f"ps3_{m}_{g}", tag="ps3")
                ps3 = grp_tiles[(m, g)]
                h2 = l23_pass(m, bt, h1c, tag)
                nc.tensor.matmul(ps3[32 * k:32 * k + Z, :],
                                 lhsT=wt[m]["wb"][:, 2 * H2:2 * H2 + Z],
                                 rhs=h2[:], start=True, stop=True,
                                 tile_position=(0, 32 * k))
                if k == gsz - 1:
                    rows = 32 * (gsz - 1) + Z
                    ot = op.tile([128, BT], F16, name=f"ot_{m}_{g}", tag="ot")
                    nc.scalar.activation(ot[0:rows, :], ps3[0:rows, :], AF.Tanh,
                                         bias=wt[m]["bf"][0:rows,
                                                          OC1 + 1:OC1 + 2])
                    # stores ride the SP queue: idle once the x stream ends,
                    # and a store here would wedge between the final tanhs on
                    # the ACT queue
                    nc.sync.dma_start(outh[m, g][0:rows, :], ot[0:rows, :])
                    after_tanh[0] = True

            # software-pipelined PE order: L1(bt+1) before L2/L3(bt).
            # bt0's accumulation groups stay open across bt1's full L1 pass so
            # x1 can be delivered before bt0's last x rows (head compression).
            warm_w1a(0)
            seq = [(m, bt) for m in range(MPC) for bt in range(NBT)]
            pend = None
            for m, bt in seq:
                if m == 1 and bt == 0:
                    warm_w1a(1)
                    warm_w1b(1)
                    warm_wb(1)
                h1c = l1_pass(m, bt, f"{m}_{bt}",
                              after_dr34=(lambda: warm_w1b(0))
                              if (m, bt) == (0, 0) else None,
                              relu_split=(m, bt) == (MPC - 1, NBT - 1))
                if pend is None:
                    warm_wb(0)
                else:
                    finish(*pend)
                pend = (m, bt, h1c)
            finish(*pend)

    nc.compile()
    return nc


def make_in_maps(x, W1, b1, W2, b2, W3, b3):
    xb = np.asarray(x, dtype=np.float32).reshape(M, B, D_IN)
    W1 = np.asarray(W1, dtype=np.float32)
    W2 = np.asarray(W2, dtype=np.float32)
    W3 = np.asarray(W3, dtype=np.float32)
    b1 = np.asarray(b1, dtype=np.float32)
    b2 = np.asarray(b2, dtype=np.float32)
    b3 = np.asarray(b3, dtype=np.float32)

    in_maps = []
    for core in range(N_CORES):
        sl = slice(core * MPC, (core + 1) * MPC)
        xr = np.ascontiguousarray(
            xb[sl].reshape(MPC, B, KC, 128).transpose(0, 3, 2, 1))
        x_hi = xr.astype(E4)
        x_lo = (xr - x_hi.astype(np.float32)).astype(E4)
        xA = np.empty((MPC, 128, XR, 2, B), dtype=E4)
        xA[:, :, 0, 0, :] = x_hi[:, :, NC_CORR, :]
        xA[:, :, 0, 1, :] = x_hi[:, :, NC_CORR + 1, :]
        for c in range(NC_CORR):
            xA[:, :, 1 + c, 0, :] = x_hi[:, :, c, :]
            xA[:, :, 1 + c, 1, :] = x_lo[:, :, c, :]

        w1r = np.ascontiguousarray(
            (32.0 * W1[sl]).reshape(MPC, H1, KC, 128).transpose(0, 3, 2, 1))
        w_hi = w1r.astype(E4)
        w_lo = (w1r - w_hi.astype(np.float32)).astype(E4)
        w1A = np.empty((MPC, 128, NW, H1), dtype=E4)
        w1A[:, :, 0, :] = w_hi[:, :, NC_CORR, :]
        w1A[:, :, 1, :] = w_hi[:, :, NC_CORR + 1, :]
        w1A[:, :, 2, :] = w_lo[:, :, NC_CORR, :]
        w1A[:, :, 3, :] = w_lo[:, :, NC_CORR + 1, :]
        for c in range(NC_CORR):
            w1A[:, :, 4 + c, :] = w_hi[:, :, c, :]
        for p in range(NPAIR):
            w1A[:, :, 10 + 2 * p, :] = w_lo[:, :, 2 * p, :]
            w1A[:, :, 11 + 2 * p, :] = w_lo[:, :, 2 * p + 1, :]

        wb = np.zeros((MPC, 128, 2 * H2 + Z), dtype=np.float16)
        w2t = (W2[sl] / 32.0).reshape(MPC, H2, 2, 128).transpose(0, 3, 2, 1)
        wb[:, :, 0:H2] = w2t[:, :, 0, :]
        wb[:, :, H2:2 * H2] = w2t[:, :, 1, :]
        wb[:, :, 2 * H2:BB] = W3[sl].transpose(0, 2, 1)
        bf = np.zeros((MPC, 128, OC1 + 2), dtype=np.float32)
        bf[:, :, 0:OC1] = (32.0 * b1[sl]).reshape(MPC, OC1, 128).transpose(0, 2, 1)
        bf[:, :, OC1] = b2[sl]
        for k in range(4):
            bf[:, 32 * k:32 * k + Z, OC1 + 1] = b3[sl]

        in_maps.append({"xh": xA, "w1h": w1A, "wbh": wb, "bfh": bf})
    return in_maps


def kernel(x, W1, b1, W2, b2, W3, b3):
    global _cached, last_results
    if _cached is None:
        _cached = build_bass()
    nc = _cached

    in_maps = make_in_maps(x, W1, b1, W2, b2, W3, b3)
    res = run_bass_kernel_spmd(nc, in_maps, list(range(N_CORES)))
    last_results = res

    out = np.empty((M, B, Z), dtype=np.float32)
    for core in range(N_CORES):
        oh = res.results[core]["outh"]
        for m in range(MPC):
            gm = core * MPC + m
            s = 0
            for g, gsz in enumerate(GROUPS[m]):
                for k in range(gsz):
                    bt = s + k
                    out[gm, bt * BT:(bt + 1) * BT, :] = (
                        oh[m, g, 32 * k:32 * k + Z, :].T.astype(np.float32))
                s += gsz
    return out


# revision 5
# speedup vs baseline: 1.0261x; 1.0042x over previous
"""Trainium2 Bass kernel: 16-member MLP ensemble (1024 -> 256 relu -> 128 relu -> 16 tanh).

Sharding: expert-parallel, 2 models per core x 8 cores, no collectives.

Layer 1 (90% of FLOPs) runs as fp8-e4m3 DoubleRow matmuls (2 k-tiles per
instruction, 0.5 cyc/col on the PE) with residual compensation:
  W' = e4m3(32*W1), Wl = e4m3(32*W1 - W'); x_hi = e4m3(x), x_lo = e4m3(x - x_hi).
  Corrected k-chunks c=0..4:  (W'_c, W'_c) . (x_hi_c, x_lo_c)        [x corrected]
  W-residual pairs:           (Wl_c0, Wl_c1) . (x_hi_c0, x_hi_c1)    [W corrected]
                              (Wl4/2, Wl4/2) . (hi4, hi4)            [odd chunk]
  Uncorrected chunks 5,6,7:   (W'5, W'6) . (hi5, hi6)
                              (W'7, Wl7) . (hi7, hi7)
                              (Wl5, Wl6) . (hi5, hi6)
  11 DoubleRows per (oc, batch-tile). Repeated pair halves are stride-0
  broadcast APs (verified on HW), and W-corr rows reuse the x_hi bytes via
  strided APs, so x costs 13/8 B/elem and W1 is sent once.
  relu(32(W1 x + b1)) = 32 relu(W1 x + b1): the 32x folds into W2/32
  host-side. Layers 2/3 fp16. L3 outputs for up to 4 batch tiles are packed
  into one PSUM bank at partition strips 0/32/64/96 (one tanh + one store per
  group; output layout [mpc, grp, 128, BT] fp16, host unpacks strips).

Schedule: every DMA is issued upfront (all x tiles stay resident in SBUF, no
WAR waits) on the SINGLE SP queue in exact consumption order -- same-queue
DMAs stream back-to-back on the DMA engines while cross-queue grant order is
unreliable. Output stores also ride SP (idle once the x stream ends). The PE
stream is software-pipelined (L1 of bt+1 before L2/L3 of bt) and warmup
matmuls absorb weight-DMA waits early so the p-state ramp completes during
the head; PE runs at its 48.0us floor with ~2e-2us of scheduling overhead.

Host-sim rel err 1.594e-2 (gate 2e-2); HW matches (1.595e-2). 60801 ns vs
85605 ns baseline (1.41x).
"""

import numpy as np
import ml_dtypes

import concourse.bacc as bacc
import concourse.bass as bass
import concourse.mybir as mybir
import concourse.tile as tile
from concourse.bass_utils import run_bass_kernel_spmd

M, B, Z = 16, 4096, 16
N_CORES = 8
MPC = M // N_CORES
D_IN, H1, H2 = 1024, 256, 128
BT = 512
NBT = B // BT
KC = D_IN // 128
NC_CORR = 5                 # k-chunks 0-4 carry an x_lo correction
OC1 = H1 // 128
XR = 2 * NC_CORR + 3        # x rows: 2c=hi_c 2c+1=lo_c (c=0..4), 10-12=hi5,hi6,hi7
NW = 16                     # w1 rows: 0-1=W'5,W'6 2-3=Wl5,Wl6 4-5=W'7,Wl7
                            #          6-10=W'c(0..4) 11-14=Wl0..Wl3 15=Wl4/2
NGRP = 3
BB = 2 * H2 + Z               # bias-column base inside the merged wb tensor

F32 = mybir.dt.float32
F16 = mybir.dt.float16
F8 = mybir.dt.float8e4
AF = mybir.ActivationFunctionType
DR = mybir.MatmulPerfMode.DoubleRow
E4 = ml_dtypes.float8_e4m3

GROUPS = [[4, 4] if m < MPC - 1 else [4, 3, 1] for m in range(MPC)]

_cached = None
last_results = None


def build_bass():
    nc = bacc.Bacc("TRN2", target_bir_lowering=False, debug=False, num_devices=N_CORES)

    xh = nc.dram_tensor("xh", [MPC, 128, XR, B], F8, kind="ExternalInput")
    w1h = nc.dram_tensor("w1h", [MPC, 128, NW, H1], F8, kind="ExternalInput")
    wbh = nc.dram_tensor("wbh", [MPC, 128, 2 * H2 + Z], F16,
                         kind="ExternalInput")
    bfh = nc.dram_tensor("bfh", [MPC, 128, OC1 + 2], F32, kind="ExternalInput")
    outh = nc.dram_tensor("outh", [MPC, NGRP, 128, BT], F16, kind="ExternalOutput")

    with tile.TileContext(nc) as tc:
        with (
            tc.tile_pool(name="static", bufs=1) as sp,
            tc.tile_pool(name="hid", bufs=6) as hp,
            tc.tile_pool(name="outs", bufs=2) as op,
            tc.tile_pool(name="ps1p", bufs=4, space="PSUM") as pp1,
            tc.tile_pool(name="ps2p", bufs=2, space="PSUM") as pp2,
            tc.tile_pool(name="ps3p", bufs=1, space="PSUM") as pp3,
            tc.tile_pool(name="warm", bufs=1, space="PSUM") as wpp,
        ):
            wt = []
            for m in range(MPC):
                wt.append({
                    "w1": sp.tile([128, NW, H1], F8, name=f"w1_{m}"),
                    "wb": sp.tile([128, 2 * H2 + Z], F16, name=f"wb_{m}"),
                    "bf": sp.tile([128, OC1 + 2], F32, name=f"bf_{m}"),
                })
            xtiles = {}
            for m in range(MPC):
                for bt in range(NBT):
                    xtiles[(m, bt)] = sp.tile([128, XR, BT], F8,
                                              name=f"x_{m}_{bt}")

            # ---- DMA issue plan ----
            def dma_x(eng, m, bt, rows=None):
                t = xtiles[(m, bt)]
                cols = slice(bt * BT, (bt + 1) * BT)
                if rows is None:
                    return eng.dma_start(t[:], xh[m][:, :, cols])
                return eng.dma_start(t[:, rows, :], xh[m][:, rows, cols])

            # Single-queue supply: same-queue DMAs stream back-to-back on the
            # DMA engines (dge/sem-prop overheads pipeline with neighbours),
            # and in-queue order is the only reliable grant order. So the
            # entire supply rides the SP queue in exact consumption order;
            # only the output stores (tiny, latency-tolerant) use the ACT
            # queue. Total transfer time ~44us < PE ~48us, so the stream
            # stays ahead of compute from bt1 on.
            nc.sync.dma_start(wt[0]["w1"][:, 0:6, :], w1h[0][:, 0:6])
            dma_x(nc.sync, 0, 0, rows=slice(10, XR))
            nc.sync.dma_start(wt[0]["w1"][:, 6:NW, :], w1h[0][:, 6:NW])
            dma_x(nc.sync, 0, 0, rows=slice(0, 6))
            dma_x(nc.sync, 0, 0, rows=slice(6, 10))
            dma_x(nc.sync, 0, 1)
            nc.sync.dma_start(wt[0]["wb"][:], wbh[0])
            nc.sync.dma_start(wt[0]["bf"][:], bfh[0])
            dma_x(nc.sync, 0, 2)
            dma_x(nc.sync, 0, 3)
            dma_x(nc.sync, 0, 4)
            dma_x(nc.sync, 0, 5)
            dma_x(nc.sync, 0, 6)
            dma_x(nc.sync, 0, 7)
            nc.sync.dma_start(wt[1]["w1"][:], w1h[1])
            nc.sync.dma_start(wt[1]["wb"][:], wbh[1])
            nc.sync.dma_start(wt[1]["bf"][:], bfh[1])
            for bt in range(7):
                dma_x(nc.sync, 1, bt)
            dma_x(nc.sync, 1, 7, rows=slice(0, 10))
            dma_x(nc.sync, 1, 7, rows=slice(10, XR))

            wps = wpp.tile([128, 16], F32, name="warm_ps")

            def warm_w1a(m):
                nc.tensor.matmul(wps[:], lhsT=wt[m]["w1"][:, 0, 0:128],
                                 rhs=wt[m]["w1"][:, 0, 0:16], start=True, stop=True)

            def warm_w1b(m):
                nc.tensor.matmul(wps[:], lhsT=wt[m]["w1"][:, 6, 0:128],
                                 rhs=wt[m]["w1"][:, 6, 0:16], start=True, stop=True)

            def warm_wb(m):
                nc.tensor.matmul(wps[:], lhsT=wt[m]["wb"][:, 0:128],
                                 rhs=wt[m]["wb"][:, 0:16], start=True, stop=True)

            def _drs(t, xt, ps1, osl, which):
                def dr1(c, stop=False, start=False):
                    nc.tensor.matmul(
                        ps1[:],
                        lhsT=t["w1"][:, 6 + c:7 + c, osl]
                            .broadcast_to([128, 2, 128]),
                        rhs=xt[:, 2 * c:2 * c + 2, :],
                        start=start, stop=stop, perf_mode=DR)

                def dr2(p, stop=False):
                    nc.tensor.matmul(
                        ps1[:], lhsT=t["w1"][:, 11 + 2 * p:13 + 2 * p, osl],
                        rhs=xt[:, 4 * p:4 * p + 3:2, :],
                        start=False, stop=stop, perf_mode=DR)

                def dr_unc():
                    # (W'5,W'6).(hi5,hi6) ; (W'7,Wl7).(hi7,hi7) ; (Wl5,Wl6).(hi5,hi6)
                    nc.tensor.matmul(ps1[:], lhsT=t["w1"][:, 0:2, osl],
                                     rhs=xt[:, 10:12, :],
                                     start=True, stop=False, perf_mode=DR)
                    nc.tensor.matmul(ps1[:], lhsT=t["w1"][:, 4:6, osl],
                                     rhs=xt[:, 12:13, :]
                                         .broadcast_to([128, 2, BT]),
                                     start=False, stop=False, perf_mode=DR)
                    nc.tensor.matmul(ps1[:], lhsT=t["w1"][:, 2:4, osl],
                                     rhs=xt[:, 10:12, :],
                                     start=False, stop=False, perf_mode=DR)

                def dr_w4(stop=False):
                    # (Wl4/2,Wl4/2).(hi4,hi4) = Wl4.hi4
                    nc.tensor.matmul(
                        ps1[:],
                        lhsT=t["w1"][:, 15:16, osl].broadcast_to([128, 2, 128]),
                        rhs=xt[:, 8:9, :].broadcast_to([128, 2, BT]),
                        start=False, stop=stop, perf_mode=DR)

                if which == "all":
                    # rows 10-12 + w1 rows 0-5 first, then corrected chunks
                    dr_unc()
                    dr1(0)
                    dr1(1)
                    dr1(2)
                    dr2(0)
                    dr1(3)
                    dr1(4)
                    dr2(1)
                    dr_w4(stop=True)
                else:
                    # tail variant: corrected chunks (x rows 0-9) first so the
                    # final row piece (10-12) leaves only 3 DRs per oc
                    dr1(0, start=True)
                    dr1(1)
                    dr1(2)
                    dr2(0)
                    dr1(3)
                    dr1(4)
                    dr2(1)
                    dr_w4()
                    nc.tensor.matmul(ps1[:], lhsT=t["w1"][:, 0:2, osl],
                                     rhs=xt[:, 10:12, :],
                                     start=False, stop=False, perf_mode=DR)
                    nc.tensor.matmul(ps1[:], lhsT=t["w1"][:, 4:6, osl],
                                     rhs=xt[:, 12:13, :]
                                         .broadcast_to([128, 2, BT]),
                                     start=False, stop=False, perf_mode=DR)
                    nc.tensor.matmul(ps1[:], lhsT=t["w1"][:, 2:4, osl],
                                     rhs=xt[:, 10:12, :],
                                     start=False, stop=True, perf_mode=DR)

            def l1_pass(m, bt, tag, after_dr34=None, tail=False):
                t = wt[m]
                xt = xtiles[(m, bt)]
                h1c = []
                for oc in range(OC1):
                    osl = slice(oc * 128, (oc + 1) * 128)
                    ps1 = pp1.tile([128, BT], F32, name=f"ps1_{tag}_{oc}",
                                   tag="ps1")
                    _drs(t, xt, ps1, osl, "tail" if tail else "all")
                    if after_dr34 is not None:
                        after_dr34()
                        after_dr34 = None
                    h1 = hp.tile([128, BT], F16, name=f"h1_{tag}_{oc}", tag="h1")
                    if tail and oc == 0:
                        nc.scalar.activation(h1[:], ps1[:], AF.Relu,
                                             bias=t["bf"][:, oc:oc + 1])
                    else:
                        nc.vector.tensor_scalar(h1[:], ps1[:],
                                                t["bf"][:, oc:oc + 1],
                                                0.0, mybir.AluOpType.add,
                                                mybir.AluOpType.max)
                    h1c.append(h1)
                return h1c

            after_tanh = [False]  # previous finish() emitted a group tanh

            def l23_pass(m, bt, h1c, tag):
                t = wt[m]
                ps2 = pp2.tile([128, BT], F32, name=f"ps2_{tag}", tag="ps2")
                for c in range(2):
                    nc.tensor.matmul(ps2[:], lhsT=t["wb"][:, c * H2:(c + 1) * H2],
                                     rhs=h1c[c][:], start=(c == 0), stop=(c == 1))
                h2 = hp.tile([128, BT], F16, name=f"h2_{tag}", tag="h2")
                if after_tanh[0]:
                    # the ACT queue is still busy with the group tanh; DVE
                    # keeps the relu (and the following L3) off that latency
                    nc.vector.tensor_scalar(h2[:], ps2[:],
                                            t["bf"][:, OC1:OC1 + 1], 0.0,
                                            mybir.AluOpType.add,
                                            mybir.AluOpType.max)
                else:
                    nc.scalar.activation(h2[:], ps2[:], AF.Relu,
                                         bias=t["bf"][:, OC1:OC1 + 1])
                after_tanh[0] = False
                return h2

            ginfo = {}
            for m in range(MPC):
                s = 0
                for g, gsz in enumerate(GROUPS[m]):
                    for k in range(gsz):
                        ginfo[(m, s + k)] = (g, k, gsz)
                    s += gsz

            grp_tiles = {}

            def finish(m, bt, h1c):
                tag = f"{m}_{bt}"
                g, k, gsz = ginfo[(m, bt)]
                if k == 0:
                    grp_tiles[(m, g)] = pp3.tile([128, BT], F32,
                                                 name=f"ps3_{m}_{g}", tag="ps3")
                ps3 = grp_tiles[(m, g)]
                h2 = l23_pass(m, bt, h1c, tag)
                nc.tensor.matmul(ps3[32 * k:32 * k + Z, :],
                                 lhsT=wt[m]["wb"][:, 2 * H2:2 * H2 + Z],
                                 rhs=h2[:], start=True, stop=True,
                                 tile_position=(0, 32 * k))
                if k == gsz - 1:
                    rows = 32 * (gsz - 1) + Z
                    ot = op.tile([128, BT], F16, name=f"ot_{m}_{g}", tag="ot")
                    nc.scalar.activation(ot[0:rows, :], ps3[0:rows, :], AF.Tanh,
                                         bias=wt[m]["bf"][0:rows,
                                                          OC1 + 1:OC1 + 2])
                    # stores ride the SP queue: idle once the x stream ends,
                    # and a store here would wedge between the final tanhs on
                    # the ACT queue
                    nc.sync.dma_start(outh[m, g][0:rows, :], ot[0:rows, :])
                    after_tanh[0] = True

            # software-pipelined PE order: L1(bt+1) before L2/L3(bt).
            # bt0's accumulation groups stay open across bt1's full L1 pass so
            # x1 can be delivered before bt0's last x rows (head compression).
            warm_w1a(0)
            seq = [(m, bt) for m in range(MPC) for bt in range(NBT)]
            pend = None
            for m, bt in seq:
                if m == 1 and bt == 0:
                    warm_w1a(1)
                    warm_w1b(1)
                    warm_wb(1)
                h1c = l1_pass(m, bt, f"{m}_{bt}",
                              after_dr34=(lambda: warm_w1b(0))
                              if (m, bt) == (0, 0) else None,
                              tail=(m, bt) == (MPC - 1, NBT - 1))
                if pend is None:
                    warm_wb(0)
                else:
                    finish(*pend)
                pend = (m, bt, h1c)
            finish(*pend)

    nc.compile()
    return nc


def make_in_maps(x, W1, b1, W2, b2, W3, b3):
    xb = np.asarray(x, dtype=np.float32).reshape(M, B, D_IN)
    W1 = np.asarray(W1, dtype=np.float32)
    W2 = np.asarray(W2, dtype=np.float32)
    W3 = np.asarray(W3, dtype=np.float32)
    b1 = np.asarray(b1, dtype=np.float32)
    b2 = np.asarray(b2, dtype=np.float32)
    b3 = np.asarray(b3, dtype=np.float32)

    in_maps = []
    for core in range(N_CORES):
        sl = slice(core * MPC, (core + 1) * MPC)
        xr = np.ascontiguousarray(
            xb[sl].reshape(MPC, B, KC, 128).transpose(0, 3, 2, 1))
        x_hi = xr.astype(E4)
        x_lo = (xr - x_hi.astype(np.float32)).astype(E4)
        xA = np.empty((MPC, 128, XR, B), dtype=E4)
        for c in range(NC_CORR):
            xA[:, :, 2 * c, :] = x_hi[:, :, c, :]
            xA[:, :, 2 * c + 1, :] = x_lo[:, :, c, :]
        for j in range(3):
            xA[:, :, 10 + j, :] = x_hi[:, :, NC_CORR + j, :]

        w1r = np.ascontiguousarray(
            (32.0 * W1[sl]).reshape(MPC, H1, KC, 128).transpose(0, 3, 2, 1))
        w_hi = w1r.astype(E4)
        w_lo = (w1r - w_hi.astype(np.float32)).astype(E4)
        w1A = np.empty((MPC, 128, NW, H1), dtype=E4)
        w1A[:, :, 0, :] = w_hi[:, :, 5, :]
        w1A[:, :, 1, :] = w_hi[:, :, 6, :]
        w1A[:, :, 2, :] = w_lo[:, :, 5, :]
        w1A[:, :, 3, :] = w_lo[:, :, 6, :]
        w1A[:, :, 4, :] = w_hi[:, :, 7, :]
        w1A[:, :, 5, :] = w_lo[:, :, 7, :]
        for c in range(NC_CORR):
            w1A[:, :, 6 + c, :] = w_hi[:, :, c, :]
        for c in range(4):
            w1A[:, :, 11 + c, :] = w_lo[:, :, c, :]
        w1A[:, :, 15, :] = (w_lo[:, :, 4, :].astype(np.float32) * 0.5).astype(E4)

        wb = np.zeros((MPC, 128, 2 * H2 + Z), dtype=np.float16)
        w2t = (W2[sl] / 32.0).reshape(MPC, H2, 2, 128).transpose(0, 3, 2, 1)
        wb[:, :, 0:H2] = w2t[:, :, 0, :]
        wb[:, :, H2:2 * H2] = w2t[:, :, 1, :]
        wb[:, :, 2 * H2:BB] = W3[sl].transpose(0, 2, 1)
        bf = np.zeros((MPC, 128, OC1 + 2), dtype=np.float32)
        bf[:, :, 0:OC1] = (32.0 * b1[sl]).reshape(MPC, OC1, 128).transpose(0, 2, 1)
        bf[:, :, OC1] = b2[sl]
        for k in range(4):
            bf[:, 32 * k:32 * k + Z, OC1 + 1] = b3[sl]

        in_maps.append({"xh": xA, "w1h": w1A, "wbh": wb, "bfh": bf})
    return in_maps


def kernel(x, W1, b1, W2, b2, W3, b3):
    global _cached, last_results
    if _cached is None:
        _cached = build_bass()
    nc = _cached

    in_maps = make_in_maps(x, W1, b1, W2, b2, W3, b3)
    res = run_bass_kernel_spmd(nc, in_maps, list(range(N_CORES)))
    last_results = res

    out = np.empty((M, B, Z), dtype=np.float32)
    for core in range(N_CORES):
        oh = res.results[core]["outh"]
        for m in range(MPC):
            gm = core * MPC + m
            s = 0
            for g, gsz in enumerate(GROUPS[m]):
                for k in range(gsz):
                    bt = s + k
                    out[gm, bt * BT:(bt + 1) * BT, :] = (
                        oh[m, g, 32 * k:32 * k + Z, :].T.astype(np.float32))
                s += gsz
    return out


# revision 7
# speedup vs baseline: 1.0403x; 1.0138x over previous
"""Trainium2 Bass kernel: 16-member MLP ensemble (1024 -> 256 relu -> 128 relu -> 16 tanh).

Sharding: expert-parallel, 2 models per core x 8 cores, no collectives.

Layer 1 (90% of FLOPs) runs as fp8-e4m3 DoubleRow matmuls (2 k-tiles per
instruction, 0.5 cyc/col on the PE) with residual compensation:
  W' = e4m3(32*W1), Wl = e4m3(32*W1 - W'); x_hi = e4m3(x), x_lo = e4m3(x - x_hi).
  Corrected k-chunks c=0..4:  (W'_c, W'_c) . (x_hi_c, x_lo_c)        [x corrected]
  W-residual pairs:           (Wl_c0, Wl_c1) . (x_hi_c0, x_hi_c1)    [W corrected]
                              (Wl4/2, Wl4/2) . (hi4, hi4)            [odd chunk]
  Uncorrected chunks 5,6,7:   (W'5, W'6) . (hi5, hi6)
                              (W'7, Wl7) . (hi7, hi7)
                              (Wl5, Wl6) . (hi5, hi6)
  11 DoubleRows per (oc, batch-tile). Repeated pair halves are stride-0
  broadcast APs (verified on HW), and W-corr rows reuse the x_hi bytes via
  strided APs, so x costs 13/8 B/elem and W1 is sent once.
  relu(32(W1 x + b1)) = 32 relu(W1 x + b1): the 32x folds into W2/32
  host-side. Layers 2/3 fp16. L3 outputs for up to 4 batch tiles are packed
  into one PSUM bank at partition strips 0/32/64/96 (one tanh + one store per
  group; output layout [mpc, grp, 128, BT] fp16, host unpacks strips).

Schedule: every DMA is issued upfront (all x tiles stay resident in SBUF, no
WAR waits) on the SINGLE SP queue in exact consumption order -- same-queue
DMAs stream back-to-back on the DMA engines while cross-queue grant order is
unreliable. Output stores also ride SP (idle once the x stream ends). The PE
stream is software-pipelined (L1 of bt+1 before L2/L3 of bt) and warmup
matmuls absorb weight-DMA waits early so the p-state ramp completes during
the head; PE runs at its 48.0us floor with ~2e-2us of scheduling overhead.

The first four batch tiles are supplied as column-halves that the host packs
contiguously (keeping >=512B DMA descriptors); their DoubleRows accumulate
into disjoint column ranges of the same PSUM bank, so L2/L3 stay full-width.

Host-sim rel err 1.594e-2 (gate 2e-2); HW matches (1.595e-2). 60545 ns vs
85605 ns baseline (1.41x).
"""

import numpy as np
import ml_dtypes

import concourse.bacc as bacc
import concourse.bass as bass
import concourse.mybir as mybir
import concourse.tile as tile
from concourse.bass_utils import run_bass_kernel_spmd

M, B, Z = 16, 4096, 16
N_CORES = 8
MPC = M // N_CORES
D_IN, H1, H2 = 1024, 256, 128
BT = 512
NBT = B // BT
KC = D_IN // 128
NC_CORR = 5                 # k-chunks 0-4 carry an x_lo correction
OC1 = H1 // 128
XR = 2 * NC_CORR + 3        # x rows: 2c=hi_c 2c+1=lo_c (c=0..4), 10-12=hi5,hi6,hi7
NW = 16                     # w1 rows: 0-1=W'5,W'6 2-3=Wl5,Wl6 4-5=W'7,Wl7
                            #          6-10=W'c(0..4) 11-14=Wl0..Wl3 15=Wl4/2
NGRP = 3
BB = 2 * H2 + Z               # bias-column base inside the merged wb tensor

F32 = mybir.dt.float32
F16 = mybir.dt.float16
F8 = mybir.dt.float8e4
AF = mybir.ActivationFunctionType
DR = mybir.MatmulPerfMode.DoubleRow
E4 = ml_dtypes.float8_e4m3

GROUPS = [[4, 4] if m < MPC - 1 else [4, 3, 1] for m in range(MPC)]

_cached = None
last_results = None


def build_bass():
    nc = bacc.Bacc("TRN2", target_bir_lowering=False, debug=False, num_devices=N_CORES)

    xh = nc.dram_tensor("xh", [MPC, 128, XR, B], F8, kind="ExternalInput")
    xh2 = nc.dram_tensor("xh2", [4, 2, 128, XR, BT // 2], F8,
                         kind="ExternalInput")
    w1h = nc.dram_tensor("w1h", [MPC, 128, NW, H1], F8, kind="ExternalInput")
    wbh = nc.dram_tensor("wbh", [MPC, 128, 2 * H2 + Z], F16,
                         kind="ExternalInput")
    bfh = nc.dram_tensor("bfh", [MPC, 128, OC1 + 2], F32, kind="ExternalInput")
    outh = nc.dram_tensor("outh", [MPC, NGRP, 128, BT], F16, kind="ExternalOutput")

    with tile.TileContext(nc) as tc:
        with (
            tc.tile_pool(name="static", bufs=1) as sp,
            tc.tile_pool(name="hid", bufs=6) as hp,
            tc.tile_pool(name="outs", bufs=2) as op,
            tc.tile_pool(name="ps1p", bufs=4, space="PSUM") as pp1,
            tc.tile_pool(name="ps2p", bufs=2, space="PSUM") as pp2,
            tc.tile_pool(name="ps3p", bufs=1, space="PSUM") as pp3,
            tc.tile_pool(name="warm", bufs=1, space="PSUM") as wpp,
        ):
            wt = []
            for m in range(MPC):
                wt.append({
                    "w1": sp.tile([128, NW, H1], F8, name=f"w1_{m}"),
                    "wb": sp.tile([128, 2 * H2 + Z], F16, name=f"wb_{m}"),
                    "bf": sp.tile([128, OC1 + 2], F32, name=f"bf_{m}"),
                })
            xtiles = {}
            for m in range(MPC):
                for bt in range(NBT):
                    if m == 0 and bt < 4:
                        continue
                    xtiles[(m, bt)] = sp.tile([128, XR, BT], F8,
                                              name=f"x_{m}_{bt}")
            # head half-tiles: each is column-contiguous in DRAM so the DMA
            # keeps >=512B descriptors despite the 256-col width
            htiles = {}
            for bt in range(4):
                for h in range(2):
                    htiles[(bt, h)] = sp.tile([128, XR, BT // 2], F8,
                                              name=f"xh_{bt}_{h}")

            # ---- DMA issue plan ----
            def dma_x(eng, m, bt, rows=None):
                t = xtiles[(m, bt)]
                cols = slice(bt * BT, (bt + 1) * BT)
                if rows is None:
                    return eng.dma_start(t[:], xh[m][:, :, cols])
                return eng.dma_start(t[:, rows, :], xh[m][:, rows, cols])

            # Single-queue supply: same-queue DMAs stream back-to-back on the
            # DMA engines (dge/sem-prop overheads pipeline with neighbours),
            # and in-queue order is the only reliable grant order. So the
            # entire supply rides the SP queue in exact consumption order;
            # only the output stores (tiny, latency-tolerant) use the ACT
            # queue. Total transfer time ~44us < PE ~48us, so the stream
            # stays ahead of compute from bt1 on.
            nc.sync.dma_start(wt[0]["w1"][:, 0:6, :], w1h[0][:, 0:6])
            nc.sync.dma_start(htiles[(0, 0)][:, 10:XR, :], xh2[0, 0][:, 10:XR])
            nc.sync.dma_start(wt[0]["w1"][:, 6:NW, :], w1h[0][:, 6:NW])
            nc.sync.dma_start(htiles[(0, 0)][:, 0:10, :], xh2[0, 0][:, 0:10])
            nc.sync.dma_start(htiles[(0, 1)][:], xh2[0, 1])
            nc.sync.dma_start(htiles[(1, 0)][:], xh2[1, 0])
            nc.sync.dma_start(htiles[(1, 1)][:], xh2[1, 1])
            nc.sync.dma_start(wt[0]["wb"][:], wbh[0])
            nc.sync.dma_start(wt[0]["bf"][:], bfh[0])
            nc.sync.dma_start(htiles[(2, 0)][:], xh2[2, 0])
            nc.sync.dma_start(htiles[(2, 1)][:], xh2[2, 1])
            nc.sync.dma_start(htiles[(3, 0)][:], xh2[3, 0])
            nc.sync.dma_start(htiles[(3, 1)][:], xh2[3, 1])
            dma_x(nc.sync, 0, 4)
            dma_x(nc.sync, 0, 5)
            dma_x(nc.sync, 0, 6)
            dma_x(nc.sync, 0, 7)
            nc.sync.dma_start(wt[1]["w1"][:], w1h[1])
            nc.sync.dma_start(wt[1]["wb"][:], wbh[1])
            nc.sync.dma_start(wt[1]["bf"][:], bfh[1])
            for bt in range(7):
                dma_x(nc.sync, 1, bt)
            dma_x(nc.sync, 1, 7, rows=slice(0, 10))
            dma_x(nc.sync, 1, 7, rows=slice(10, XR))

            wps = wpp.tile([128, 16], F32, name="warm_ps")

            def warm_w1a(m):
                nc.tensor.matmul(wps[:], lhsT=wt[m]["w1"][:, 0, 0:128],
                                 rhs=wt[m]["w1"][:, 0, 0:16], start=True, stop=True)

            def warm_w1b(m):
                nc.tensor.matmul(wps[:], lhsT=wt[m]["w1"][:, 6, 0:128],
                                 rhs=wt[m]["w1"][:, 6, 0:16], start=True, stop=True)

            def warm_wb(m):
                nc.tensor.matmul(wps[:], lhsT=wt[m]["wb"][:, 0:128],
                                 rhs=wt[m]["wb"][:, 0:16], start=True, stop=True)

            def _drs(t, xt, ps1, osl, which, out_cols=slice(0, BT), width=BT):
                out = ps1[:, out_cols]

                def dr1(c, stop=False, start=False):
                    nc.tensor.matmul(
                        out,
                        lhsT=t["w1"][:, 6 + c:7 + c, osl]
                            .broadcast_to([128, 2, 128]),
                        rhs=xt[:, 2 * c:2 * c + 2, :],
                        start=start, stop=stop, perf_mode=DR)

                def dr2(p, stop=False):
                    nc.tensor.matmul(
                        out, lhsT=t["w1"][:, 11 + 2 * p:13 + 2 * p, osl],
                        rhs=xt[:, 4 * p:4 * p + 3:2, :],
                        start=False, stop=stop, perf_mode=DR)

                def dr_unc():
                    # (W'5,W'6).(hi5,hi6) ; (W'7,Wl7).(hi7,hi7) ; (Wl5,Wl6).(hi5,hi6)
                    nc.tensor.matmul(out, lhsT=t["w1"][:, 0:2, osl],
                                     rhs=xt[:, 10:12, :],
                                     start=True, stop=False, perf_mode=DR)
                    nc.tensor.matmul(out, lhsT=t["w1"][:, 4:6, osl],
                                     rhs=xt[:, 12:13, :]
                                         .broadcast_to([128, 2, width]),
                                     start=False, stop=False, perf_mode=DR)
                    nc.tensor.matmul(out, lhsT=t["w1"][:, 2:4, osl],
                                     rhs=xt[:, 10:12, :],
                                     start=False, stop=False, perf_mode=DR)

                def dr_w4(stop=False):
                    # (Wl4/2,Wl4/2).(hi4,hi4) = Wl4.hi4
                    nc.tensor.matmul(
                        out,
                        lhsT=t["w1"][:, 15:16, osl].broadcast_to([128, 2, 128]),
                        rhs=xt[:, 8:9, :].broadcast_to([128, 2, width]),
                        start=False, stop=stop, perf_mode=DR)

                if which == "all":
                    # rows 10-12 + w1 rows 0-5 first, then corrected chunks
                    dr_unc()
                    dr1(0)
                    dr1(1)
                    dr1(2)
                    dr2(0)
                    dr1(3)
                    dr1(4)
                    dr2(1)
                    dr_w4(stop=True)
                else:
                    # tail variant: corrected chunks (x rows 0-9) first so the
                    # final row piece (10-12) leaves only 3 DRs per oc
                    dr1(0, start=True)
                    dr1(1)
                    dr1(2)
                    dr2(0)
                    dr1(3)
                    dr1(4)
                    dr2(1)
                    dr_w4()
                    nc.tensor.matmul(out, lhsT=t["w1"][:, 0:2, osl],
                                     rhs=xt[:, 10:12, :],
                                     start=False, stop=False, perf_mode=DR)
                    nc.tensor.matmul(out, lhsT=t["w1"][:, 4:6, osl],
                                     rhs=xt[:, 12:13, :]
                                         .broadcast_to([128, 2, width]),
                                     start=False, stop=False, perf_mode=DR)
                    nc.tensor.matmul(out, lhsT=t["w1"][:, 2:4, osl],
                                     rhs=xt[:, 10:12, :],
                                     start=False, stop=True, perf_mode=DR)

            def l1_pass_halved(bt, tag, after_dr34=None):
                t = wt[0]
                ps1s = [pp1.tile([128, BT], F32, name=f"ps1_{tag}_{oc}",
                                 tag="ps1") for oc in range(OC1)]
                for h in range(2):
                    xt = htiles[(bt, h)]
                    csl = slice(h * (BT // 2), (h + 1) * (BT // 2))
                    for oc in range(OC1):
                        _drs(t, xt, ps1s[oc], slice(oc * 128, (oc + 1) * 128),
                             "all", out_cols=csl, width=BT // 2)
                        if after_dr34 is not None:
                            after_dr34()
                            after_dr34 = None
                h1c = []
                for oc in range(OC1):
                    h1 = hp.tile([128, BT], F16, name=f"h1_{tag}_{oc}", tag="h1")
                    nc.vector.tensor_scalar(h1[:], ps1s[oc][:],
                                            t["bf"][:, oc:oc + 1],
                                            0.0, mybir.AluOpType.add,
                                            mybir.AluOpType.max)
                    h1c.append(h1)
                return h1c

            def l1_pass(m, bt, tag, after_dr34=None, tail=False):
                t = wt[m]
                xt = xtiles[(m, bt)]
                h1c = []
                for oc in range(OC1):
                    osl = slice(oc * 128, (oc + 1) * 128)
                    ps1 = pp1.tile([128, BT], F32, name=f"ps1_{tag}_{oc}",
                                   tag="ps1")
                    _drs(t, xt, ps1, osl, "tail" if tail else "all")
                    if after_dr34 is not None:
                        after_dr34()
                        after_dr34 = None
                    h1 = hp.tile([128, BT], F16, name=f"h1_{tag}_{oc}", tag="h1")
                    if tail and oc == 0:
                        nc.scalar.activation(h1[:], ps1[:], AF.Relu,
                                             bias=t["bf"][:, oc:oc + 1])
                    else:
                        nc.vector.tensor_scalar(h1[:], ps1[:],
                                                t["bf"][:, oc:oc + 1],
                                                0.0, mybir.AluOpType.add,
                                                mybir.AluOpType.max)
                    h1c.append(h1)
                return h1c

            after_tanh = [False]  # previous finish() emitted a group tanh

            def l23_pass(m, bt, h1c, tag):
                t = wt[m]
                ps2 = pp2.tile([128, BT], F32, name=f"ps2_{tag}", tag="ps2")
                for c in range(2):
                    nc.tensor.matmul(ps2[:], lhsT=t["wb"][:, c * H2:(c + 1) * H2],
                                     rhs=h1c[c][:], start=(c == 0), stop=(c == 1))
                h2 = hp.tile([128, BT], F16, name=f"h2_{tag}", tag="h2")
                if after_tanh[0]:
                    # the ACT queue is still busy with the group tanh; DVE
                    # keeps the relu (and the following L3) off that latency
                    nc.vector.tensor_scalar(h2[:], ps2[:],
                                            t["bf"][:, OC1:OC1 + 1], 0.0,
                                            mybir.AluOpType.add,
                                            mybir.AluOpType.max)
                else:
                    nc.scalar.activation(h2[:], ps2[:], AF.Relu,
                                         bias=t["bf"][:, OC1:OC1 + 1])
                after_tanh[0] = False
                return h2

            ginfo = {}
            for m in range(MPC):
                s = 0
                for g, gsz in enumerate(GROUPS[m]):
                    for k in range(gsz):
                        ginfo[(m, s + k)] = (g, k, gsz)
                    s += gsz

            grp_tiles = {}

            def finish(m, bt, h1c):
                tag = f"{m}_{bt}"
                g, k, gsz = ginfo[(m, bt)]
                if k == 0:
                    grp_tiles[(m, g)] = pp3.tile([128, BT], F32,
                                                 name=f"ps3_{m}_{g}", tag="ps3")
                ps3 = grp_tiles[(m, g)]
                h2 = l23_pass(m, bt, h1c, tag)
                nc.tensor.matmul(ps3[32 * k:32 * k + Z, :],
                                 lhsT=wt[m]["wb"][:, 2 * H2:2 * H2 + Z],
                                 rhs=h2[:], start=True, stop=True,
                                 tile_position=(0, 32 * k))
                if k == gsz - 1:
                    rows = 32 * (gsz - 1) + Z
                    ot = op.tile([128, BT], F16, name=f"ot_{m}_{g}", tag="ot")
                    nc.scalar.activation(ot[0:rows, :], ps3[0:rows, :], AF.Tanh,
                                         bias=wt[m]["bf"][0:rows,
                                                          OC1 + 1:OC1 + 2])
                    # stores ride the SP queue: idle once the x stream ends,
                    # and a store here would wedge between the final tanhs on
                    # the ACT queue
                    nc.sync.dma_start(outh[m, g][0:rows, :], ot[0:rows, :])
                    after_tanh[0] = True

            # software-pipelined PE order: L1(bt+1) before L2/L3(bt).
            # bt0's accumulation groups stay open across bt1's full L1 pass so
            # x1 can be delivered before bt0's last x rows (head compression).
            warm_w1a(0)
            seq = [(m, bt) for m in range(MPC) for bt in range(NBT)]
            pend = None
            for m, bt in seq:
                if m == 1 and bt == 0:
                    warm_w1a(1)
                    warm_w1b(1)
                    warm_wb(1)
                if m == 0 and bt < 4:
                    h1c = l1_pass_halved(bt, f"{m}_{bt}",
                                         after_dr34=(lambda: warm_w1b(0))
                                         if bt == 0 else None)
                else:
                    h1c = l1_pass(m, bt, f"{m}_{bt}",
                                  tail=(m, bt) == (MPC - 1, NBT - 1))
                if pend is None:
                    warm_wb(0)
                else:
                    finish(*pend)
                pend = (m, bt, h1c)
            finish(*pend)

    nc.compile()
    return nc


def make_in_maps(x, W1, b1, W2, b2, W3, b3):
    xb = np.asarray(x, dtype=np.float32).reshape(M, B, D_IN)
    W1 = np.asarray(W1, dtype=np.float32)
    W2 = np.asarray(W2, dtype=np.float32)
    W3 = np.asarray(W3, dtype=np.float32)
    b1 = np.asarray(b1, dtype=np.float32)
    b2 = np.asarray(b2, dtype=np.float32)
    b3 = np.asarray(b3, dtype=np.float32)

    in_maps = []
    for core in range(N_CORES):
        sl = slice(core * MPC, (core + 1) * MPC)
        xr = np.ascontiguousarray(
            xb[sl].reshape(MPC, B, KC, 128).transpose(0, 3, 2, 1))
        x_hi = xr.astype(E4)
        x_lo = (xr - x_hi.astype(np.float32)).astype(E4)
        xA = np.empty((MPC, 128, XR, B), dtype=E4)
        for c in range(NC_CORR):
            xA[:, :, 2 * c, :] = x_hi[:, :, c, :]
            xA[:, :, 2 * c + 1, :] = x_lo[:, :, c, :]
        for j in range(3):
            xA[:, :, 10 + j, :] = x_hi[:, :, NC_CORR + j, :]

        w1r = np.ascontiguousarray(
            (32.0 * W1[sl]).reshape(MPC, H1, KC, 128).transpose(0, 3, 2, 1))
        w_hi = w1r.astype(E4)
        w_lo = (w1r - w_hi.astype(np.float32)).astype(E4)
        w1A = np.empty((MPC, 128, NW, H1), dtype=E4)
        w1A[:, :, 0, :] = w_hi[:, :, 5, :]
        w1A[:, :, 1, :] = w_hi[:, :, 6, :]
        w1A[:, :, 2, :] = w_lo[:, :, 5, :]
        w1A[:, :, 3, :] = w_lo[:, :, 6, :]
        w1A[:, :, 4, :] = w_hi[:, :, 7, :]
        w1A[:, :, 5, :] = w_lo[:, :, 7, :]
        for c in range(NC_CORR):
            w1A[:, :, 6 + c, :] = w_hi[:, :, c, :]
        for c in range(4):
            w1A[:, :, 11 + c, :] = w_lo[:, :, c, :]
        w1A[:, :, 15, :] = (w_lo[:, :, 4, :].astype(np.float32) * 0.5).astype(E4)

        wb = np.zeros((MPC, 128, 2 * H2 + Z), dtype=np.float16)
        w2t = (W2[sl] / 32.0).reshape(MPC, H2, 2, 128).transpose(0, 3, 2, 1)
        wb[:, :, 0:H2] = w2t[:, :, 0, :]
        wb[:, :, H2:2 * H2] = w2t[:, :, 1, :]
        wb[:, :, 2 * H2:BB] = W3[sl].transpose(0, 2, 1)
        bf = np.zeros((MPC, 128, OC1 + 2), dtype=np.float32)
        bf[:, :, 0:OC1] = (32.0 * b1[sl]).reshape(MPC, OC1, 128).transpose(0, 2, 1)
        bf[:, :, OC1] = b2[sl]
        for k in range(4):
            bf[:, 32 * k:32 * k + Z, OC1 + 1] = b3[sl]

        xB2 = np.empty((4, 2, 128, XR, BT // 2), dtype=E4)
        for bt in range(4):
            for h in range(2):
                c0 = bt * BT + h * (BT // 2)
                xB2[bt, h] = xA[0, :, :, c0:c0 + BT // 2]
        in_maps.append({"xh": xA, "xh2": xB2, "w1h": w1A, "wbh": wb,
                        "bfh": bf})
    return in_maps


def kernel(x, W1, b1, W2, b2, W3, b3):
    global _cached, last_results
    if _cached is None:
        _cached = build_bass()
    nc = _cached

    in_maps = make_in_maps(x, W1, b1, W2, b2, W3, b3)
    res = run_bass_kernel_spmd(nc, in_maps, list(range(N_CORES)))
    last_results = res

    out = np.empty((M, B, Z), dtype=np.float32)
    for core in range(N_CORES):
        oh = res.results[core]["outh"]
        for m in range(MPC):
            gm = core * MPC + m
            s = 0
            for g, gsz in enumerate(GROUPS[m]):
                for k in range(gsz):
                    bt = s + k
                    out[gm, bt * BT:(bt + 1) * BT, :] = (
                        oh[m, g, 32 * k:32 * k + Z, :].T.astype(np.float32))
                s += gsz
    return out


# revision 8
# speedup vs baseline: 1.0417x; 1.0014x over previous
"""Trainium2 Bass kernel: 16-member MLP ensemble (1024 -> 256 relu -> 128 relu -> 16 tanh).

Sharding: expert-parallel, 2 models per core x 8 cores, no collectives.

Layer 1 (90% of FLOPs) runs as fp8-e4m3 DoubleRow matmuls (2 k-tiles per
instruction, 0.5 cyc/col on the PE) with residual compensation:
  W' = e4m3(32*W1), Wl = e4m3(32*W1 - W'); x_hi = e4m3(x), x_lo = e4m3(x - x_hi).
  Corrected k-chunks c=0..4:  (W'_c, W'_c) . (x_hi_c, x_lo_c)        [x corrected]
  W-residual pairs:           (Wl_c0, Wl_c1) . (x_hi_c0, x_hi_c1)    [W corrected]
                              (Wl4/2, Wl4/2) . (hi4, hi4)            [odd chunk]
  Uncorrected chunks 5,6,7:   (W'5, W'6) . (hi5, hi6)
                              (W'7, Wl7) . (hi7, hi7)
                              (Wl5, Wl6) . (hi5, hi6)
  11 DoubleRows per (oc, batch-tile). Repeated pair halves are stride-0
  broadcast APs (verified on HW), and W-corr rows reuse the x_hi bytes via
  strided APs, so x costs 13/8 B/elem and W1 is sent once.
  relu(32(W1 x + b1)) = 32 relu(W1 x + b1): the 32x folds into W2/32
  host-side. Layers 2/3 fp16. L3 outputs for up to 4 batch tiles are packed
  into one PSUM bank at partition strips 0/32/64/96 (one tanh + one store per
  group; output layout [mpc, grp, 128, BT] fp16, host unpacks strips).

Schedule: every DMA is issued upfront (all x tiles stay resident in SBUF, no
WAR waits) on the SINGLE SP queue in exact consumption order -- same-queue
DMAs stream back-to-back on the DMA engines while cross-queue grant order is
unreliable. Output stores also ride SP (idle once the x stream ends). The PE
stream is software-pipelined (L1 of bt+1 before L2/L3 of bt) and warmup
matmuls absorb weight-DMA waits early so the p-state ramp completes during
the head; PE runs at its 48.0us floor with ~2e-2us of scheduling overhead.

The first four batch tiles are supplied as column-halves that the host packs
contiguously (keeping >=512B DMA descriptors); their DoubleRows accumulate
into disjoint column ranges of the same PSUM bank, so L2/L3 stay full-width.

Host-sim rel err 1.594e-2 (gate 2e-2); HW matches (1.595e-2). 60545 ns vs
85605 ns baseline (1.41x).
"""

import numpy as np
import ml_dtypes

import concourse.bacc as bacc
import concourse.bass as bass
import concourse.mybir as mybir
import concourse.tile as tile
from concourse.bass_utils import run_bass_kernel_spmd

M, B, Z = 16, 4096, 16
N_CORES = 8
MPC = M // N_CORES
D_IN, H1, H2 = 1024, 256, 128
BT = 512
NBT = B // BT
KC = D_IN // 128
NC_CORR = 5                 # k-chunks 0-4 carry an x_lo correction
OC1 = H1 // 128
XR = 2 * NC_CORR + 3        # x rows: 2c=hi_c 2c+1=lo_c (c=0..4), 10-12=hi5,hi6,hi7
NW = 16                     # w1 rows: 0-1=W'5,W'6 2-3=Wl5,Wl6 4-5=W'7,Wl7
                            #          6-10=W'c(0..4) 11-14=Wl0..Wl3 15=Wl4/2
NGRP = 3
BB = 2 * H2 + Z               # bias-column base inside the merged wb tensor

F32 = mybir.dt.float32
F16 = mybir.dt.float16
F8 = mybir.dt.float8e4
AF = mybir.ActivationFunctionType
DR = mybir.MatmulPerfMode.DoubleRow
E4 = ml_dtypes.float8_e4m3

GROUPS = [[4, 4] if m < MPC - 1 else [4, 3, 1] for m in range(MPC)]

_cached = None
last_results = None


def build_bass():
    nc = bacc.Bacc("TRN2", target_bir_lowering=False, debug=False, num_devices=N_CORES)

    xh = nc.dram_tensor("xh", [MPC, 128, XR, B], F8, kind="ExternalInput")
    xh2 = nc.dram_tensor("xh2", [4, 2, 128, XR, BT // 2], F8,
                         kind="ExternalInput")
    w1h = nc.dram_tensor("w1h", [MPC, 128, NW, H1], F8, kind="ExternalInput")
    wbh = nc.dram_tensor("wbh", [MPC, 128, 2 * H2 + Z], F16,
                         kind="ExternalInput")
    bfh = nc.dram_tensor("bfh", [MPC, 128, OC1 + 2], F32, kind="ExternalInput")
    outh = nc.dram_tensor("outh", [MPC, NGRP, 128, BT], F16, kind="ExternalOutput")

    with tile.TileContext(nc) as tc:
        with (
            tc.tile_pool(name="static", bufs=1) as sp,
            tc.tile_pool(name="hid", bufs=6) as hp,
            tc.tile_pool(name="outs", bufs=2) as op,
            tc.tile_pool(name="ps1p", bufs=4, space="PSUM") as pp1,
            tc.tile_pool(name="ps2p", bufs=2, space="PSUM") as pp2,
            tc.tile_pool(name="ps3p", bufs=1, space="PSUM") as pp3,
            tc.tile_pool(name="warm", bufs=1, space="PSUM") as wpp,
        ):
            wt = []
            for m in range(MPC):
                wt.append({
                    "w1": sp.tile([128, NW, H1], F8, name=f"w1_{m}"),
                    "wb": sp.tile([128, 2 * H2 + Z], F16, name=f"wb_{m}"),
                    "bf": sp.tile([128, OC1 + 2], F32, name=f"bf_{m}"),
                })
            xtiles = {}
            for m in range(MPC):
                for bt in range(NBT):
                    if m == 0 and bt < 4:
                        continue
                    xtiles[(m, bt)] = sp.tile([128, XR, BT], F8,
                                              name=f"x_{m}_{bt}")
            # head half-tiles: each is column-contiguous in DRAM so the DMA
            # keeps >=512B descriptors despite the 256-col width
            htiles = {}
            for bt in range(4):
                for h in range(2):
                    htiles[(bt, h)] = sp.tile([128, XR, BT // 2], F8,
                                              name=f"xh_{bt}_{h}")

            # ---- DMA issue plan ----
            def dma_x(eng, m, bt, rows=None):
                t = xtiles[(m, bt)]
                cols = slice(bt * BT, (bt + 1) * BT)
                if rows is None:
                    return eng.dma_start(t[:], xh[m][:, :, cols])
                return eng.dma_start(t[:, rows, :], xh[m][:, rows, cols])

            # Single-queue supply: same-queue DMAs stream back-to-back on the
            # DMA engines (dge/sem-prop overheads pipeline with neighbours),
            # and in-queue order is the only reliable grant order. So the
            # entire supply rides the SP queue in exact consumption order;
            # only the output stores (tiny, latency-tolerant) use the ACT
            # queue. Total transfer time ~44us < PE ~48us, so the stream
            # stays ahead of compute from bt1 on.
            nc.sync.dma_start(wt[0]["w1"][:, 0:6, :], w1h[0][:, 0:6])
            nc.sync.dma_start(htiles[(0, 0)][:, 10:XR, :], xh2[0, 0][:, 10:XR])
            nc.sync.dma_start(wt[0]["w1"][:, 6:NW, :], w1h[0][:, 6:NW])
            nc.sync.dma_start(htiles[(0, 0)][:, 0:10, :], xh2[0, 0][:, 0:10])
            nc.sync.dma_start(htiles[(0, 1)][:], xh2[0, 1])
            nc.sync.dma_start(htiles[(1, 0)][:], xh2[1, 0])
            nc.sync.dma_start(htiles[(1, 1)][:], xh2[1, 1])
            nc.sync.dma_start(wt[0]["wb"][:], wbh[0])
            nc.sync.dma_start(wt[0]["bf"][:], bfh[0])
            nc.sync.dma_start(htiles[(2, 0)][:], xh2[2, 0])
            nc.sync.dma_start(htiles[(2, 1)][:], xh2[2, 1])
            nc.sync.dma_start(htiles[(3, 0)][:], xh2[3, 0])
            nc.sync.dma_start(htiles[(3, 1)][:], xh2[3, 1])
            dma_x(nc.sync, 0, 4)
            dma_x(nc.sync, 0, 5)
            dma_x(nc.sync, 0, 6)
            dma_x(nc.sync, 0, 7)
            nc.sync.dma_start(wt[1]["w1"][:], w1h[1])
            nc.sync.dma_start(wt[1]["wb"][:], wbh[1])
            nc.sync.dma_start(wt[1]["bf"][:], bfh[1])
            for bt in range(7):
                dma_x(nc.sync, 1, bt)
            dma_x(nc.sync, 1, 7, rows=slice(0, 10))
            dma_x(nc.sync, 1, 7, rows=slice(10, XR))

            wps = wpp.tile([128, 16], F32, name="warm_ps")

            def warm_w1a(m):
                nc.tensor.matmul(wps[:], lhsT=wt[m]["w1"][:, 0, 0:128],
                                 rhs=wt[m]["w1"][:, 0, 0:16], start=True, stop=True)

            def warm_w1b(m):
                nc.tensor.matmul(wps[:], lhsT=wt[m]["w1"][:, 6, 0:128],
                                 rhs=wt[m]["w1"][:, 6, 0:16], start=True, stop=True)

            def warm_wb(m):
                nc.tensor.matmul(wps[:], lhsT=wt[m]["wb"][:, 0:128],
                                 rhs=wt[m]["wb"][:, 0:16], start=True, stop=True)

            def _drs(t, xt, ps1, osl, which, out_cols=slice(0, BT), width=BT):
                out = ps1[:, out_cols]

                def dr1(c, stop=False, start=False):
                    nc.tensor.matmul(
                        out,
                        lhsT=t["w1"][:, 6 + c:7 + c, osl]
                            .broadcast_to([128, 2, 128]),
                        rhs=xt[:, 2 * c:2 * c + 2, :],
                        start=start, stop=stop, perf_mode=DR)

                def dr2(p, stop=False):
                    nc.tensor.matmul(
                        out, lhsT=t["w1"][:, 11 + 2 * p:13 + 2 * p, osl],
                        rhs=xt[:, 4 * p:4 * p + 3:2, :],
                        start=False, stop=stop, perf_mode=DR)

                def dr_unc():
                    # (W'5,W'6).(hi5,hi6) ; (W'7,Wl7).(hi7,hi7) ; (Wl5,Wl6).(hi5,hi6)
                    nc.tensor.matmul(out, lhsT=t["w1"][:, 0:2, osl],
                                     rhs=xt[:, 10:12, :],
                                     start=True, stop=False, perf_mode=DR)
                    nc.tensor.matmul(out, lhsT=t["w1"][:, 4:6, osl],
                                     rhs=xt[:, 12:13, :]
                                         .broadcast_to([128, 2, width]),
                                     start=False, stop=False, perf_mode=DR)
                    nc.tensor.matmul(out, lhsT=t["w1"][:, 2:4, osl],
                                     rhs=xt[:, 10:12, :],
                                     start=False, stop=False, perf_mode=DR)

                def dr_w4(stop=False):
                    # (Wl4/2,Wl4/2).(hi4,hi4) = Wl4.hi4
                    nc.tensor.matmul(
                        out,
                        lhsT=t["w1"][:, 15:16, osl].broadcast_to([128, 2, 128]),
                        rhs=xt[:, 8:9, :].broadcast_to([128, 2, width]),
                        start=False, stop=stop, perf_mode=DR)

                if which == "all":
                    # rows 10-12 + w1 rows 0-5 first, then corrected chunks
                    dr_unc()
                    dr1(0)
                    dr1(1)
                    dr1(2)
                    dr2(0)
                    dr1(3)
                    dr1(4)
                    dr2(1)
                    dr_w4(stop=True)
                else:
                    # tail variant: corrected chunks (x rows 0-9) first so the
                    # final row piece (10-12) leaves only 3 DRs per oc
                    dr1(0, start=True)
                    dr1(1)
                    dr1(2)
                    dr2(0)
                    dr1(3)
                    dr1(4)
                    dr2(1)
                    dr_w4()
                    nc.tensor.matmul(out, lhsT=t["w1"][:, 0:2, osl],
                                     rhs=xt[:, 10:12, :],
                                     start=False, stop=False, perf_mode=DR)
                    nc.tensor.matmul(out, lhsT=t["w1"][:, 4:6, osl],
                                     rhs=xt[:, 12:13, :]
                                         .broadcast_to([128, 2, width]),
                                     start=False, stop=False, perf_mode=DR)
                    nc.tensor.matmul(out, lhsT=t["w1"][:, 2:4, osl],
                                     rhs=xt[:, 10:12, :],
                                     start=False, stop=True, perf_mode=DR)

            def l1_pass_halved(bt, tag, after_dr34=None):
                t = wt[0]
                ps1s = [pp1.tile([128, BT], F32, name=f"ps1_{tag}_{oc}",
                                 tag="ps1") for oc in range(OC1)]
                for h in range(2):
                    xt = htiles[(bt, h)]
                    csl = slice(h * (BT // 2), (h + 1) * (BT // 2))
                    for oc in range(OC1):
                        _drs(t, xt, ps1s[oc], slice(oc * 128, (oc + 1) * 128),
                             "all", out_cols=csl, width=BT // 2)
                        if after_dr34 is not None:
                            after_dr34()
                            after_dr34 = None
                h1c = []
                for oc in range(OC1):
                    h1 = hp.tile([128, BT], F16, name=f"h1_{tag}_{oc}", tag="h1")
                    nc.vector.tensor_scalar(h1[:], ps1s[oc][:],
                                            t["bf"][:, oc:oc + 1],
                                            0.0, mybir.AluOpType.add,
                                            mybir.AluOpType.max)
                    h1c.append(h1)
                return h1c

            def l1_pass(m, bt, tag, after_dr34=None, tail=False):
                t = wt[m]
                xt = xtiles[(m, bt)]
                h1c = []
                for oc in range(OC1):
                    osl = slice(oc * 128, (oc + 1) * 128)
                    ps1 = pp1.tile([128, BT], F32, name=f"ps1_{tag}_{oc}",
                                   tag="ps1")
                    _drs(t, xt, ps1, osl, "tail" if tail else "all")
                    if after_dr34 is not None:
                        after_dr34()
                        after_dr34 = None
                    h1 = hp.tile([128, BT], F16, name=f"h1_{tag}_{oc}", tag="h1")
                    if tail and oc == 0:
                        nc.scalar.activation(h1[:], ps1[:], AF.Relu,
                                             bias=t["bf"][:, oc:oc + 1])
                    else:
                        nc.vector.tensor_scalar(h1[:], ps1[:],
                                                t["bf"][:, oc:oc + 1],
                                                0.0, mybir.AluOpType.add,
                                                mybir.AluOpType.max)
                    h1c.append(h1)
                return h1c

            after_tanh = [False]  # previous finish() emitted a group tanh

            def l23_pass(m, bt, h1c, tag):
                t = wt[m]
                ps2 = pp2.tile([128, BT], F32, name=f"ps2_{tag}", tag="ps2")
                for c in range(2):
                    nc.tensor.matmul(ps2[:], lhsT=t["wb"][:, c * H2:(c + 1) * H2],
                                     rhs=h1c[c][:], start=(c == 0), stop=(c == 1))
                h2 = hp.tile([128, BT], F16, name=f"h2_{tag}", tag="h2")
                if after_tanh[0]:
                    # the ACT queue is still busy with the group tanh; DVE
                    # keeps the relu (and the following L3) off that latency
                    nc.vector.tensor_scalar(h2[:], ps2[:],
                                            t["bf"][:, OC1:OC1 + 1], 0.0,
                                            mybir.AluOpType.add,
                                            mybir.AluOpType.max)
                else:
                    nc.scalar.activation(h2[:], ps2[:], AF.Relu,
                                         bias=t["bf"][:, OC1:OC1 + 1])
                after_tanh[0] = False
                return h2

            ginfo = {}
            for m in range(MPC):
                s = 0
                for g, gsz in enumerate(GROUPS[m]):
                    for k in range(gsz):
                        ginfo[(m, s + k)] = (g, k, gsz)
                    s += gsz

            grp_tiles = {}

            def fin2(m, bt, h1c):
                """L2 matmuls + h2 relu for a batch tile."""
                return l23_pass(m, bt, h1c, f"{m}_{bt}")

            def fin3(m, bt, h2):
                """L3 strip matmul (+ group tanh/store) for a batch tile."""
                g, k, gsz = ginfo[(m, bt)]
                if k == 0:
                    grp_tiles[(m, g)] = pp3.tile([128, BT], F32,
                                                 name=f"ps3_{m}_{g}", tag="ps3")
                ps3 = grp_tiles[(m, g)]
                nc.tensor.matmul(ps3[32 * k:32 * k + Z, :],
                                 lhsT=wt[m]["wb"][:, 2 * H2:2 * H2 + Z],
                                 rhs=h2[:], start=True, stop=True,
                                 tile_position=(0, 32 * k))
                if k == gsz - 1:
                    rows = 32 * (gsz - 1) + Z
                    ot = op.tile([128, BT], F16, name=f"ot_{m}_{g}", tag="ot")
                    nc.scalar.activation(ot[0:rows, :], ps3[0:rows, :], AF.Tanh,
                                         bias=wt[m]["bf"][0:rows,
                                                          OC1 + 1:OC1 + 2])
                    # stores ride the SP queue: idle once the x stream ends,
                    # and a store here would wedge between the final tanhs on
                    # the ACT queue
                    nc.sync.dma_start(outh[m, g][0:rows, :], ot[0:rows, :])
                    after_tanh[0] = True

            # software-pipelined PE order: L1(bt+1) before L2/L3(bt).
            # bt0's accumulation groups stay open across bt1's full L1 pass so
            # x1 can be delivered before bt0's last x rows (head compression).
            # two-stage software pipeline: per iteration emit
            # [L1(bt), L2(bt-1), L3(bt-2)] so both the h1-relu chain (feeding
            # L2) and the h2-relu chain (feeding L3) get a full L1 window on
            # the in-order PE queue
            warm_w1a(0)
            seq = [(m, bt) for m in range(MPC) for bt in range(NBT)]
            p1 = None  # (m, bt, h1c) awaiting L2
            p2 = None  # (m, bt, h2) awaiting L3
            for m, bt in seq:
                if m == 1 and bt == 0:
                    warm_w1a(1)
                    warm_w1b(1)
                    warm_wb(1)
                if m == 0 and bt < 4:
                    h1c = l1_pass_halved(bt, f"{m}_{bt}",
                                         after_dr34=(lambda: warm_w1b(0))
                                         if bt == 0 else None)
                else:
                    h1c = l1_pass(m, bt, f"{m}_{bt}",
                                  tail=(m, bt) == (MPC - 1, NBT - 1))
                if p1 is None:
                    warm_wb(0)
                else:
                    h2 = fin2(*p1)
                    if p2 is not None:
                        fin3(*p2)
                    p2 = (p1[0], p1[1], h2)
                p1 = (m, bt, h1c)
            h2 = fin2(*p1)
            fin3(*p2)
            fin3(p1[0], p1[1], h2)

    nc.compile()
    return nc


def make_in_maps(x, W1, b1, W2, b2, W3, b3):
    xb = np.asarray(x, dtype=np.float32).reshape(M, B, D_IN)
    W1 = np.asarray(W1, dtype=np.float32)
    W2 = np.asarray(W2, dtype=np.float32)
    W3 = np.asarray(W3, dtype=np.float32)
    b1 = np.asarray(b1, dtype=np.float32)
    b2 = np.asarray(b2, dtype=np.float32)
    b3 = np.asarray(b3, dtype=np.float32)

    in_maps = []
    for core in range(N_CORES):
        sl = slice(core * MPC, (core + 1) * MPC)
        xr = np.ascontiguousarray(
            xb[sl].reshape(MPC, B, KC, 128).transpose(0, 3, 2, 1))
        x_hi = xr.astype(E4)
        x_lo = (xr - x_hi.astype(np.float32)).astype(E4)
        xA = np.empty((MPC, 128, XR, B), dtype=E4)
        for c in range(NC_CORR):
            xA[:, :, 2 * c, :] = x_hi[:, :, c, :]
            xA[:, :, 2 * c + 1, :] = x_lo[:, :, c, :]
        for j in range(3):
            xA[:, :, 10 + j, :] = x_hi[:, :, NC_CORR + j, :]

        w1r = np.ascontiguousarray(
            (32.0 * W1[sl]).reshape(MPC, H1, KC, 128).transpose(0, 3, 2, 1))
        w_hi = w1r.astype(E4)
        w_lo = (w1r - w_hi.astype(np.float32)).astype(E4)
        w1A = np.empty((MPC, 128, NW, H1), dtype=E4)
        w1A[:, :, 0, :] = w_hi[:, :, 5, :]
        w1A[:, :, 1, :] = w_hi[:, :, 6, :]
        w1A[:, :, 2, :] = w_lo[:, :, 5, :]
        w1A[:, :, 3, :] = w_lo[:, :, 6, :]
        w1A[:, :, 4, :] = w_hi[:, :, 7, :]
        w1A[:, :, 5, :] = w_lo[:, :, 7, :]
        for c in range(NC_CORR):
            w1A[:, :, 6 + c, :] = w_hi[:, :, c, :]
        for c in range(4):
            w1A[:, :, 11 + c, :] = w_lo[:, :, c, :]
        w1A[:, :, 15, :] = (w_lo[:, :, 4, :].astype(np.float32) * 0.5).astype(E4)

        wb = np.zeros((MPC, 128, 2 * H2 + Z), dtype=np.float16)
        w2t = (W2[sl] / 32.0).reshape(MPC, H2, 2, 128).transpose(0, 3, 2, 1)
        wb[:, :, 0:H2] = w2t[:, :, 0, :]
        wb[:, :, H2:2 * H2] = w2t[:, :, 1, :]
        wb[:, :, 2 * H2:BB] = W3[sl].transpose(0, 2, 1)
        bf = np.zeros((MPC, 128, OC1 + 2), dtype=np.float32)
        bf[:, :, 0:OC1] = (32.0 * b1[sl]).reshape(MPC, OC1, 128).transpose(0, 2, 1)
        bf[:, :, OC1] = b2[sl]
        for k in range(4):
            bf[:, 32 * k:32 * k + Z, OC1 + 1] = b3[sl]

        xB2 = np.empty((4, 2, 128, XR, BT // 2), dtype=E4)
        for bt in range(4):
            for h in range(2):
                c0 = bt * BT + h * (BT // 2)
                xB2[bt, h] = xA[0, :, :, c0:c0 + BT // 2]
        in_maps.append({"xh": xA, "xh2": xB2, "w1h": w1A, "wbh": wb,
                        "bfh": bf})
    return in_maps


def kernel(x, W1, b1, W2, b2, W3, b3):
    global _cached, last_results
    if _cached is None:
        _cached = build_bass()
    nc = _cached

    in_maps = make_in_maps(x, W1, b1, W2, b2, W3, b3)
    res = run_bass_kernel_spmd(nc, in_maps, list(range(N_CORES)))
    last_results = res

    out = np.empty((M, B, Z), dtype=np.float32)
    for core in range(N_CORES):
        oh = res.results[core]["outh"]
        for m in range(MPC):
            gm = core * MPC + m
            s = 0
            for g, gsz in enumerate(GROUPS[m]):
                for k in range(gsz):
                    bt = s + k
                    out[gm, bt * BT:(bt + 1) * BT, :] = (
                        oh[m, g, 32 * k:32 * k + Z, :].T.astype(np.float32))
                s += gsz
    return out
